# revision 17
# baseline (speedup 1.0000x reference)
"""Trainium2 Bass kernel for hybrid location-sensitive attention.

Problem: nn_AttentionMechanism_54752243089428
  keys  = enc @ W_enc + b_enc                       [B,T,A]
  query = dec @ W_dec + b_dec                       [B,1,A]
  conv  = Conv1d(attn_prev, conv_w) + conv_b        [B,10,T]
  cfeat = conv^T @ W_fil + b_fil                    [B,T,A]
  energy= tanh(keys+query+cfeat) @ v_a (+b_va)      [B,T]
  attn  = softmax(energy, axis=T)
  ctx   = sum_t attn * enc                          [B,1,E]
  returns (ctx, attn)

Strategy (8 NeuronCores, data-parallel over batch, 4 batches/core):
 - All big compute in bf16 (rel-err gate is 2e-2; bf16 keeps ~0.5%).
 - Layout: [A/E on partitions, T on free].  enc is cast f32->bf16 during
   DMA (SWDGE) into [t,e] staging tiles, then one xbar DMA-transpose per
   512-t quarter produces xt tiles laid out [p, j, ec, f] (e=128*ec+p,
   t=512*q+128*j+f).  All transposes issue from the Sync engine only
   (concurrent xbar use from two HWDGE engines races on xbar state).
 - Conv is folded into the keys matmul: conv_feat^T = M^T xs where
   M[k,a] = sum_c conv_w[c,k] W_fil[c,a] (precomputed on PE) and
   xs[k,t] = xpad[t+k] is a shifted-window matrix built by one strided
   DMA from a padded copy of attn_prev in scratch DRAM.  Row 0 is ones,
   paired with a per-batch row qb = dec@W_dec + (all biases), so one
   PSUM accumulation group produces keys+query+conv_feat+biases.
 - tanh on ScalarE (PSUM->SBUF), energy = v^T tanh via M=1 matmuls,
   softmax without max-subtraction (|energy| <= sum|v| ~ 11, exp safe),
   exp+sum fused via activation accum_out.  t is processed in four
   512 chunks; t in [2000, 2048) is zero-padded and masked out of exp.
 - context via scalar_tensor_tensor (fused multiply + free-dim reduce)
   on VectorE against a PE-broadcast normalized-attn row.  b_va shifts
   all energies equally -> cancels in softmax -> ignored.
"""

import os
import sys

sys.path.insert(0, "/opt/trn_rl_repo")

KVAR = os.environ.get("KVAR", "full")

import numpy as np

import concourse.bass as bass
import concourse.mybir as mybir
import concourse.tile as tile
from concourse import bacc
from concourse.bass_utils import run_bass_kernel_spmd

FP32 = mybir.dt.float32
BF16 = mybir.dt.bfloat16
ALU = mybir.AluOpType
AF = mybir.ActivationFunctionType
AX = mybir.AxisListType

B, T, E, A = 32, 2000, 512, 512
OUT_CH, TAPS, PADK = 10, 101, 50
NCORES = 8
BL = B // NCORES  # 4 batches per core
KX = TAPS + 1  # ones row + taps
TP = 2048  # padded T (t in [2000,2048) zeroed/masked)
XPW = 2176  # padded attn_prev row width (>= TP + PADK + TAPS)
NE = E // 128  # 4 e-chunks
NA = A // 128  # 4 a-chunks
NQ = 4  # t quarters of 512
TVAL = [512, 512, 512, 464]  # valid t per quarter


def build_nc():
    nc = bacc.Bacc("TRN2", target_bir_lowering=False)

    enc = nc.declare_dram_parameter("encoder_states", [BL, T, E], FP32, isOutput=False)
    dec = nc.declare_dram_parameter("decoder_outputs", [BL, 1, E], FP32, isOutput=False)
    awt = nc.declare_dram_parameter("attention_weights_step", [BL, T], FP32, isOutput=False)
    w_enc = nc.declare_dram_parameter("W_enc", [E, A], FP32, isOutput=False)
    b_enc = nc.declare_dram_parameter("b_enc", [A], FP32, isOutput=False)
    w_dec = nc.declare_dram_parameter("W_dec", [E, A], FP32, isOutput=False)
    b_dec = nc.declare_dram_parameter("b_dec", [A], FP32, isOutput=False)
    conv_w = nc.declare_dram_parameter("conv_w", [OUT_CH, 1, TAPS], FP32, isOutput=False)
    conv_b = nc.declare_dram_parameter("conv_b", [OUT_CH], FP32, isOutput=False)
    w_fil = nc.declare_dram_parameter("W_fil", [OUT_CH, A], FP32, isOutput=False)
    b_fil = nc.declare_dram_parameter("b_fil", [A], FP32, isOutput=False)
    v_a = nc.declare_dram_parameter("v_a", [A], FP32, isOutput=False)

    out_attn = nc.declare_dram_parameter("out_attn", [BL, T], FP32, isOutput=True)
    out_ctx = nc.declare_dram_parameter("out_ctx", [BL * NE, 128], FP32, isOutput=True)

    with tile.TileContext(nc) as tc:
        with (
            tc.tile_pool(name="wts", bufs=1) as wts,
            tc.tile_pool(name="xt", bufs=16) as xtp,
            tc.tile_pool(name="stg", bufs=5) as stgp,
            tc.tile_pool(name="stgf", bufs=3) as stgfp,
            tc.tile_pool(name="xs", bufs=4) as xsp,
            tc.tile_pool(name="mext", bufs=4) as mxp,
            tc.tile_pool(name="tanh", bufs=8) as thp,
            tc.tile_pool(name="rows", bufs=3) as rows,
            tc.tile_pool(name="ctx", bufs=2) as ctxp,
            tc.tile_pool(name="psA", bufs=3, space="PSUM") as psA,
            tc.tile_pool(name="psB", bufs=3, space="PSUM") as psB,
            tc.tile_pool(name="psC", bufs=2, space="PSUM") as psC,
            tc.tile_pool(name="dram", bufs=1, space="DRAM") as dramp,
        ):
            # ---------------- weights / constants ----------------
            # small loads first so the critical-path chain (vd32 -> vdT ->
            # qm -> mext, M, biases) is not queued behind megabyte loads
            wfil_sb = wts.tile([OUT_CH, A], BF16, tag="wfil")
            nc.gpsimd.dma_start(wfil_sb[:], w_fil[:, :])
            cw_sb = wts.tile([OUT_CH, KX], BF16, tag="cw")
            nc.vector.memset(cw_sb[:], 0.0)
            nc.gpsimd.dma_start(cw_sb[:, 1 : 1 + TAPS], conv_w[:, 0, :])
            cb_sb = wts.tile([OUT_CH, 1], BF16, tag="cb")
            nc.gpsimd.dma_start(cb_sb[:], conv_b.rearrange("(p o) -> p o", o=1))

            ber = wts.tile([1, A], FP32, tag="ber")
            nc.scalar.dma_start(ber[:], b_enc[None, :])
            bdr = wts.tile([1, A], FP32, tag="bdr")
            nc.scalar.dma_start(bdr[:], b_dec[None, :])
            bfr = wts.tile([1, A], FP32, tag="bfr")
            nc.scalar.dma_start(bfr[:], b_fil[None, :])

            # v_a and decoder outputs, transposed to column layout via DVE
            vd32 = wts.tile([32, E], BF16, tag="vd32")
            nc.vector.memset(vd32[:], 0.0)
            nc.gpsimd.dma_start(vd32[0:1, :], v_a[None, :])
            nc.gpsimd.dma_start(vd32[8 : 8 + BL, :], dec[:, 0, :])

            wenc_sb = []
            wdec_sb = []

            def emit_w_loads():
                for ec in range(NE):
                    wf = stgfp.tile([128, A], FP32, tag="wf32", name=f"wef{ec}")
                    nc.scalar.dma_start(wf[:], w_enc[ec * 128 : (ec + 1) * 128, :])
                    t_we = wts.tile([128, A], BF16, tag=f"wenc{ec}", name=f"we{ec}")
                    nc.gpsimd.tensor_copy(t_we[:], wf[:])
                    wenc_sb.append(t_we)
                for ec in range(NE):
                    wf = stgfp.tile([128, A], FP32, tag="wf32", name=f"wdf{ec}")
                    nc.scalar.dma_start(wf[:], w_dec[ec * 128 : (ec + 1) * 128, :])
                    t_wd = wts.tile([128, A], BF16, tag=f"wdec{ec}", name=f"wd{ec}")
                    nc.gpsimd.tensor_copy(t_wd[:], wf[:])
                    wdec_sb.append(t_wd)
            vdT = []
            for ec in range(NE):
                t_vdT = wts.tile([128, 32], BF16, tag=f"vdT{ec}")
                for bi in range(4):
                    nc.vector.transpose(
                        t_vdT[32 * bi : 32 * (bi + 1), :],
                        vd32[:, ec * 128 + 32 * bi : ec * 128 + 32 * (bi + 1)],
                    )
                vdT.append(t_vdT)

            ones_bf = wts.tile([1, 128], BF16, tag="ones")
            nc.vector.memset(ones_bf[:], 1.0)

            # M = conv_w^T @ W_fil (row 0 zero); cbW = conv_b @ W_fil
            m_ps = psA.tile([KX, A], FP32, tag="main")
            nc.tensor.matmul(m_ps[:], lhsT=cw_sb[:], rhs=wfil_sb[:], start=True, stop=True)
            m_base = wts.tile([KX, A], BF16, tag="mbase")
            nc.scalar.copy(m_base[:], m_ps[:])
            cbw_ps = psC.tile([1, A], FP32, tag="small")
            nc.tensor.matmul(cbw_ps[:], lhsT=cb_sb[:], rhs=wfil_sb[:], start=True, stop=True)
            bconst = wts.tile([1, A], FP32, tag="bconst")
            nc.vector.tensor_add(bconst[:], ber[:], bdr[:])
            nc.vector.tensor_add(bconst[:], bconst[:], bfr[:])
            nc.vector.tensor_add(bconst[:], bconst[:], cbw_ps[:])

            # ---------------- padded attn_prev in scratch DRAM ----------------
            xpad = dramp.tile([BL, XPW], BF16)
            zedge = wts.tile([BL, 128], BF16, tag="zedge")
            nc.vector.memset(zedge[:], 0.0)
            nc.scalar.dma_start(xpad[:, 0:PADK], zedge[:, 0:PADK])
            nc.scalar.dma_start(xpad[:, PADK + T : XPW], zedge[:, 0 : XPW - PADK - T])
            nc.gpsimd.dma_start(xpad[:, PADK : PADK + T], awt[:, :])

            junk = wts.tile([128, 512], BF16, tag="junk")
            ctx_all = wts.tile([128, 32], FP32, tag="ctxall")
            nc.vector.memset(ctx_all[:], 0.0)

            # ---------------- per-batch stages ----------------
            def emit_enc_loads(b):
                """enc -> bf16 staging -> one xbar transpose per quarter."""
                xqs = []
                for q in range(NQ):
                    xq = xtp.tile([128, 4, NE, 128], BF16, tag="xt", name=f"xt_b{b}q{q}")
                    xqs.append(xq)
                # t in [2000, 2048) of quarter 3 must be zero (not NaN)
                nc.vector.memset(xqs[3][:, 3, :, 80:128], 0.0)
                for q in range(3):
                    stgf = stgfp.tile([128, 4, 512], FP32, tag="stgf")
                    nc.scalar.dma_start(
                        stgf[:],
                        enc[b, q * 512 : (q + 1) * 512, :].rearrange(
                            "(j p) e -> p j e", p=128
                        ),
                    )
                    stg = stgp.tile([128, 4, 512], BF16, tag="stg")
                    nc.gpsimd.tensor_copy(stg[:], stgf[:])
                    nc.sync.dma_start(
                        xqs[q][:].rearrange("p j c f -> p (j c) f"),
                        stg[:].rearrange("p j e -> p (j e)"),
                        transpose=True,
                    )
                stgf3 = stgfp.tile([128, 3, 512], FP32, tag="stgf")
                nc.scalar.dma_start(
                    stgf3[:],
                    enc[b, 1536:1920, :].rearrange("(j p) e -> p j e", p=128),
                )
                stg3 = stgp.tile([128, 3, 512], BF16, tag="stg")
                nc.gpsimd.tensor_copy(stg3[:], stgf3[:])
                nc.sync.dma_start(
                    xqs[3][:, 0:3, :, :].rearrange("p j c f -> p (j c) f"),
                    stg3[:].rearrange("p j e -> p (j e)"),
                    transpose=True,
                )
                tailf = stgfp.tile([80, 512], FP32, tag="tailf", bufs=2)
                nc.scalar.dma_start(tailf[:], enc[b, 1920:2000, :])
                tail = stgp.tile([80, 512], BF16, tag="tail")
                nc.gpsimd.tensor_copy(tail[:], tailf[:])
                nc.sync.dma_start(xqs[3][:, 3, :, 0:80], tail[:], transpose=True)
                return {"xqs": xqs}

            def emit_aux_loads(b, st):
                """shifted-window matrix + per-batch bias row."""
                xs = xsp.tile([KX, TP], BF16, tag="xs", name=f"xs{b}")
                nc.vector.memset(xs[:], 1.0)
                base = xpad[:, :]
                win = bass.AP(base.tensor, base.offset + b * XPW, [[1, TAPS], [1, TP]])
                nc.scalar.dma_start(xs[1 : 1 + TAPS, :], win)

                qm_ps = psC.tile([1, A], FP32, tag="small", name=f"qm{b}")
                for ec in range(NE):
                    nc.tensor.matmul(
                        qm_ps[:],
                        lhsT=vdT[ec][:, 8 + b : 9 + b],
                        rhs=wdec_sb[ec][:],
                        start=(ec == 0),
                        stop=(ec == NE - 1),
                    )
                mext = mxp.tile([KX, A], BF16, tag="mext", name=f"mext{b}")
                nc.vector.tensor_copy(mext[:], m_base[:])
                nc.vector.scalar_tensor_tensor(
                    out=mext[0:1, :],
                    in0=qm_ps[:],
                    scalar=1.0,
                    in1=bconst[:],
                    op0=ALU.mult,
                    op1=ALU.add,
                )
                st["xs"] = xs
                st["mext"] = mext

            def emit_main(b, st):
                """keys+conv+bias matmuls, tanh, energy, exp(+partial sums)."""
                xqs, xs, mext = st["xqs"], st["xs"], st["mext"]
                exp_b = rows.tile([1, TP], BF16, tag="exp", bufs=2)
                nc.vector.memset(exp_b[:], 0.0)
                zparts = rows.tile([1, 4], FP32, tag="zp")
                st["exp"] = exp_b
                st["zp"] = zparts
                for q in range(NQ):
                    t0 = q * 512
                    ths = []
                    for ac in range(NA):
                        ps = psA.tile([128, 512], FP32, tag="main")
                        for ec in range(NE):
                            nc.tensor.matmul(
                                ps[:],
                                lhsT=wenc_sb[ec][:, ac * 128 : (ac + 1) * 128],
                                rhs=xqs[q][:, :, ec, :],
                                start=(ec == 0),
                                stop=False,
                            )
                        nc.tensor.matmul(
                            ps[:],
                            lhsT=mext[:, ac * 128 : (ac + 1) * 128],
                            rhs=xs[:, t0 : t0 + 512],
                            start=False,
                            stop=True,
                        )
                        th = thp.tile([128, 512], BF16, tag="tanh")
                        nc.scalar.activation(th[:], ps[:], AF.Tanh)
                        ths.append(th)
                    en_ps = psC.tile([1, 512], FP32, tag="small")
                    for ac in range(NA):
                        nc.tensor.matmul(
                            en_ps[:],
                            lhsT=vdT[ac][:, 0:1],
                            rhs=ths[ac][:],
                            start=(ac == 0),
                            stop=(ac == NA - 1),
                        )
                    tv = TVAL[q]
                    nc.scalar.activation(
                        exp_b[0:1, t0 : t0 + tv],
                        en_ps[0:1, 0:tv],
                        AF.Exp,
                        accum_out=zparts[0:1, q : q + 1],
                    )

            def emit_softmax(b, st):
                exp_b, zparts = st["exp"], st["zp"]
                zsum = rows.tile([1, 1], FP32, tag="zs")
                nc.vector.tensor_reduce(zsum[:], zparts[:], AX.X, ALU.add)
                recip = rows.tile([1, 1], FP32, tag="rc")
                nc.vector.reciprocal(recip[:], zsum[:])
                attn32 = rows.tile([1, T], FP32, tag="attn32", bufs=2)
                for q in range(NQ):
                    t0 = q * 512
                    tv = TVAL[q]
                    nc.scalar.activation(
                        attn32[0:1, t0 : t0 + tv],
                        exp_b[0:1, t0 : t0 + tv],
                        AF.Copy,
                        scale=recip[:],
                    )
                    nc.scalar.activation(
                        exp_b[0:1, t0 : t0 + tv],
                        exp_b[0:1, t0 : t0 + tv],
                        AF.Copy,
                        scale=recip[:],
                    )
                nc.scalar.dma_start(out_attn[b : b + 1, :], attn32[:])

            def emit_context(b, st):
                xqs, exp_b = st["xqs"], st["exp"]
                ctx_acc = ctxp.tile([128, 16], FP32, tag="ctxacc")
                for q in range(NQ):
                    t0 = q * 512
                    bc_ps = psB.tile([128, 512], FP32, tag="bcast")
                    nc.tensor.matmul(
                        bc_ps[:],
                        lhsT=ones_bf[:],
                        rhs=exp_b[0:1, t0 : t0 + 512],
                        start=True,
                        stop=True,
                    )
                    for ec in range(NE):
                        nc.vector.scalar_tensor_tensor(
                            out=junk[:],
                            in0=xqs[q][:, :, ec, :],
                            scalar=1.0,
                            in1=bc_ps[:],
                            op0=ALU.mult,
                            op1=ALU.mult,
                            accum_out=ctx_acc[:, ec * 4 + q : ec * 4 + q + 1],
                        )
                nc.vector.tensor_reduce(
                    ctx_all[:, b * 4 : (b + 1) * 4],
                    ctx_acc[:].rearrange("p (a g) -> p a g", g=4),
                    AX.X,
                    ALU.add,
                )

            # pipeline: prefetch loads one batch ahead; softmax+context of
            # batch b run right after main(b) so PE's bcast matmuls are not
            # queued behind main(b+1), and DVE context work overlaps it.
            states = {0: emit_enc_loads(0)}
            emit_w_loads()
            emit_aux_loads(0, states[0])
            states[1] = emit_enc_loads(1)
            emit_aux_loads(1, states[1])
            for b in range(BL):
                if b + 2 < BL:
                    states[b + 2] = emit_enc_loads(b + 2)
                    emit_aux_loads(b + 2, states[b + 2])
                emit_main(b, states[b])
                emit_softmax(b, states[b])
                emit_context(b, states[b])

            # context out: transpose [128 e, 16 (b,ec)] -> [16, 128]
            ctxT = wts.tile([32, 128], FP32, tag="ctxT")
            for bi in range(4):
                nc.vector.transpose(
                    ctxT[0:32, 32 * bi : 32 * (bi + 1)],
                    ctx_all[32 * bi : 32 * (bi + 1), 0:32],
                )
            nc.scalar.dma_start(out_ctx[:, :], ctxT[0 : BL * NE, :])

    nc.compile()
    return nc


_NC = None


def _get_nc():
    global _NC
    if _NC is None:
        _NC = build_nc()
    return _NC


def kernel(**inputs):
    nc = _get_nc()
    full = {k: np.ascontiguousarray(np.asarray(v, np.float32)) for k, v in inputs.items()}
    in_maps = []
    for i in range(NCORES):
        sl = slice(i * BL, (i + 1) * BL)
        m = {
            "encoder_states": full["encoder_states"][sl],
            "decoder_outputs": full["decoder_outputs"][sl],
            "attention_weights_step": full["attention_weights_step"][sl],
        }
        for k in ("W_enc", "b_enc", "W_dec", "b_dec", "conv_w", "conv_b", "W_fil", "b_fil", "v_a"):
            m[k] = full[k]
        in_maps.append(m)
    res = run_bass_kernel_spmd(nc, in_maps, core_ids=list(range(NCORES)))
    attn = np.concatenate([r["out_attn"] for r in res.results], axis=0)
    ctx = np.concatenate(
        [r["out_ctx"].reshape(BL, E)[:, None, :] for r in res.results], axis=0
    )
    return (ctx.astype(np.float32), attn.astype(np.float32))


# revision 22
# speedup vs baseline: 1.0655x; 1.0655x over previous
"""Trainium2 Bass kernel for hybrid location-sensitive attention.

Problem: nn_AttentionMechanism_54752243089428
  keys  = enc @ W_enc + b_enc                       [B,T,A]
  query = dec @ W_dec + b_dec                       [B,1,A]
  conv  = Conv1d(attn_prev, conv_w) + conv_b        [B,10,T]
  cfeat = conv^T @ W_fil + b_fil                    [B,T,A]
  energy= tanh(keys+query+cfeat) @ v_a (+b_va)      [B,T]
  attn  = softmax(energy, axis=T)
  ctx   = sum_t attn * enc                          [B,1,E]
  returns (ctx, attn)

Strategy (8 NeuronCores, data-parallel over batch, 4 batches/core):
 - All big compute in bf16 (rel-err gate is 2e-2; bf16 keeps ~0.5%).
 - Layout: [A/E on partitions, T on free].  enc is cast f32->bf16 during
   DMA (SWDGE) into [t,e] staging tiles, then one xbar DMA-transpose per
   512-t quarter produces xt tiles laid out [p, j, ec, f] (e=128*ec+p,
   t=512*q+128*j+f).  All transposes issue from the Sync engine only
   (concurrent xbar use from two HWDGE engines races on xbar state).
 - Conv is folded into the keys matmul: conv_feat^T = M^T xs where
   M[k,a] = sum_c conv_w[c,k] W_fil[c,a] (precomputed on PE) and
   xs[k,t] = xpad[t+k] is a shifted-window matrix built by one strided
   DMA from a padded copy of attn_prev in scratch DRAM.  Row 0 is ones,
   paired with a per-batch row qb = dec@W_dec + (all biases), so one
   PSUM accumulation group produces keys+query+conv_feat+biases.
 - tanh on ScalarE (PSUM->SBUF), energy = v^T tanh via M=1 matmuls,
   softmax without max-subtraction (|energy| <= sum|v| ~ 11, exp safe),
   exp+sum fused via activation accum_out.  t is processed in four
   512 chunks; t in [2000, 2048) is zero-padded and masked out of exp.
 - context via scalar_tensor_tensor (fused multiply + free-dim reduce)
   on VectorE against a PE-broadcast normalized-attn row.  b_va shifts
   all energies equally -> cancels in softmax -> ignored.
"""

import os
import sys

sys.path.insert(0, "/opt/trn_rl_repo")

KVAR = os.environ.get("KVAR", "full")

import numpy as np

import concourse.bass as bass
import concourse.mybir as mybir
import concourse.tile as tile
from concourse import bacc
from concourse.bass_utils import run_bass_kernel_spmd

FP32 = mybir.dt.float32
BF16 = mybir.dt.bfloat16
ALU = mybir.AluOpType
AF = mybir.ActivationFunctionType
AX = mybir.AxisListType

B, T, E, A = 32, 2000, 512, 512
OUT_CH, TAPS, PADK = 10, 101, 50
NCORES = 8
BL = B // NCORES  # 4 batches per core
KX = TAPS + 1  # ones row + taps
TP = 2048  # padded T (t in [2000,2048) zeroed/masked)
XPW = 2176  # padded attn_prev row width (>= TP + PADK + TAPS)
NE = E // 128  # 4 e-chunks
NA = A // 128  # 4 a-chunks
NQ = 4  # t quarters of 512
TVAL = [512, 512, 512, 464]  # valid t per quarter


def build_nc():
    nc = bacc.Bacc("TRN2", target_bir_lowering=False)

    enc = nc.declare_dram_parameter("encoder_states", [BL, T, E], FP32, isOutput=False)
    dec = nc.declare_dram_parameter("decoder_outputs", [BL, 1, E], FP32, isOutput=False)
    awt = nc.declare_dram_parameter("attention_weights_step", [BL, T], FP32, isOutput=False)
    w_enc = nc.declare_dram_parameter("W_enc", [E, A], FP32, isOutput=False)
    b_enc = nc.declare_dram_parameter("b_enc", [A], FP32, isOutput=False)
    w_dec = nc.declare_dram_parameter("W_dec", [E, A], FP32, isOutput=False)
    b_dec = nc.declare_dram_parameter("b_dec", [A], FP32, isOutput=False)
    conv_w = nc.declare_dram_parameter("conv_w", [OUT_CH, 1, TAPS], FP32, isOutput=False)
    conv_b = nc.declare_dram_parameter("conv_b", [OUT_CH], FP32, isOutput=False)
    w_fil = nc.declare_dram_parameter("W_fil", [OUT_CH, A], FP32, isOutput=False)
    b_fil = nc.declare_dram_parameter("b_fil", [A], FP32, isOutput=False)
    v_a = nc.declare_dram_parameter("v_a", [A], FP32, isOutput=False)

    out_attn = nc.declare_dram_parameter("out_attn", [BL, T], FP32, isOutput=True)
    out_ctx = nc.declare_dram_parameter("out_ctx", [BL * NE, 128], FP32, isOutput=True)

    with tile.TileContext(nc) as tc:
        with (
            tc.tile_pool(name="wts", bufs=1) as wts,
            tc.tile_pool(name="xt", bufs=16) as xtp,
            tc.tile_pool(name="stg", bufs=6) as stgp,
            tc.tile_pool(name="xs", bufs=4) as xsp,
            tc.tile_pool(name="mext", bufs=4) as mxp,
            tc.tile_pool(name="tanh", bufs=8) as thp,
            tc.tile_pool(name="rows", bufs=3) as rows,
            tc.tile_pool(name="ctx", bufs=2) as ctxp,
            tc.tile_pool(name="psA", bufs=3, space="PSUM") as psA,
            tc.tile_pool(name="psB", bufs=3, space="PSUM") as psB,
            tc.tile_pool(name="psC", bufs=2, space="PSUM") as psC,
            tc.tile_pool(name="dram", bufs=1, space="DRAM") as dramp,
        ):
            # ---------------- weights / constants ----------------
            # small loads first so the critical-path chain (vd32 -> vdT ->
            # qm -> mext, M, biases) is not queued behind megabyte loads
            wfil_sb = wts.tile([OUT_CH, A], BF16, tag="wfil")
            nc.gpsimd.dma_start(wfil_sb[:], w_fil[:, :])
            cw_sb = wts.tile([OUT_CH, KX], BF16, tag="cw")
            nc.vector.memset(cw_sb[:], 0.0)
            nc.gpsimd.dma_start(cw_sb[:, 1 : 1 + TAPS], conv_w[:, 0, :])
            cb_sb = wts.tile([OUT_CH, 1], BF16, tag="cb")
            nc.gpsimd.dma_start(cb_sb[:], conv_b.rearrange("(p o) -> p o", o=1))

            ber = wts.tile([1, A], FP32, tag="ber")
            nc.scalar.dma_start(ber[:], b_enc[None, :])
            bdr = wts.tile([1, A], FP32, tag="bdr")
            nc.scalar.dma_start(bdr[:], b_dec[None, :])
            bfr = wts.tile([1, A], FP32, tag="bfr")
            nc.scalar.dma_start(bfr[:], b_fil[None, :])

            # v_a and decoder outputs, transposed to column layout via DVE
            vd32 = wts.tile([32, E], BF16, tag="vd32")
            nc.vector.memset(vd32[:], 0.0)
            nc.gpsimd.dma_start(vd32[0:1, :], v_a[None, :])
            nc.gpsimd.dma_start(vd32[8 : 8 + BL, :], dec[:, 0, :])

            wenc_sb = []
            wdec_sb = []

            def emit_w_loads():
                for ec in range(NE):
                    t_we = wts.tile([128, A], BF16, tag=f"wenc{ec}", name=f"we{ec}")
                    nc.gpsimd.dma_start(t_we[:], w_enc[ec * 128 : (ec + 1) * 128, :])
                    wenc_sb.append(t_we)
                for ec in range(NE):
                    t_wd = wts.tile([128, A], BF16, tag=f"wdec{ec}", name=f"wd{ec}")
                    nc.gpsimd.dma_start(t_wd[:], w_dec[ec * 128 : (ec + 1) * 128, :])
                    wdec_sb.append(t_wd)
            vdT = []
            for ec in range(NE):
                t_vdT = wts.tile([128, 32], BF16, tag=f"vdT{ec}")
                for bi in range(4):
                    nc.vector.transpose(
                        t_vdT[32 * bi : 32 * (bi + 1), :],
                        vd32[:, ec * 128 + 32 * bi : ec * 128 + 32 * (bi + 1)],
                    )
                vdT.append(t_vdT)

            ones_bf = wts.tile([1, 128], BF16, tag="ones")
            nc.vector.memset(ones_bf[:], 1.0)

            # M = conv_w^T @ W_fil (row 0 zero); cbW = conv_b @ W_fil
            m_ps = psA.tile([KX, A], FP32, tag="main")
            nc.tensor.matmul(m_ps[:], lhsT=cw_sb[:], rhs=wfil_sb[:], start=True, stop=True)
            m_base = wts.tile([KX, A], BF16, tag="mbase")
            nc.scalar.copy(m_base[:], m_ps[:])
            cbw_ps = psC.tile([1, A], FP32, tag="small")
            nc.tensor.matmul(cbw_ps[:], lhsT=cb_sb[:], rhs=wfil_sb[:], start=True, stop=True)
            bconst = wts.tile([1, A], FP32, tag="bconst")
            nc.vector.tensor_add(bconst[:], ber[:], bdr[:])
            nc.vector.tensor_add(bconst[:], bconst[:], bfr[:])
            nc.vector.tensor_add(bconst[:], bconst[:], cbw_ps[:])

            # ---------------- padded attn_prev in scratch DRAM ----------------
            xpad = dramp.tile([BL, XPW], BF16)
            zedge = wts.tile([BL, 128], BF16, tag="zedge")
            nc.vector.memset(zedge[:], 0.0)
            nc.scalar.dma_start(xpad[:, 0:PADK], zedge[:, 0:PADK])
            nc.scalar.dma_start(xpad[:, PADK + T : XPW], zedge[:, 0 : XPW - PADK - T])
            nc.gpsimd.dma_start(xpad[:, PADK : PADK + T], awt[:, :])

            junk = wts.tile([128, 512], BF16, tag="junk")
            ctx_all = wts.tile([128, 32], FP32, tag="ctxall")
            nc.vector.memset(ctx_all[:], 0.0)

            # ---------------- per-batch stages ----------------
            def emit_enc_loads(b):
                """enc -> bf16 staging (partition p holds 4 consecutive t
                rows -> one 8 KB contiguous read per partition) -> one xbar
                transpose per 512-t quarter.  Resulting xt column order
                within a quarter is t = 4f + j (consistently permuted)."""
                xqs = []
                for q in range(NQ):
                    xq = xtp.tile([128, 4, NE, 128], BF16, tag="xt", name=f"xt_b{b}q{q}")
                    xqs.append(xq)
                for q in range(3):
                    stg = stgp.tile([128, 2048], BF16, tag="stg")
                    nc.gpsimd.dma_start(
                        stg[:],
                        enc[b, q * 512 : (q + 1) * 512, :].rearrange(
                            "(p j) e -> p (j e)", j=4
                        ),
                    )
                    nc.sync.dma_start(
                        xqs[q][:].rearrange("p j c f -> p (j c) f"),
                        stg[:],
                        transpose=True,
                    )
                # quarter 3: valid t rows 1536..2000 = 116 partitions * 4;
                # zero partitions 116.. so t in [2000,2048) transposes to 0
                stg3 = stgp.tile([128, 2048], BF16, tag="stg")
                # 32-aligned partition base; rows 96:116 are overwritten by
                # the load below, rows 116:128 stay zero
                nc.vector.memset(stg3[96:128, :], 0.0)
                nc.gpsimd.dma_start(
                    stg3[0:116, :],
                    enc[b, 1536:2000, :].rearrange("(p j) e -> p (j e)", j=4),
                )
                nc.sync.dma_start(
                    xqs[3][:].rearrange("p j c f -> p (j c) f"),
                    stg3[:],
                    transpose=True,
                )
                return {"xqs": xqs}

            def emit_aux_loads(b, st):
                """shifted-window matrix (+ column-permuted copy) and the
                per-batch bias row."""
                xs = xsp.tile([KX, TP], BF16, tag="xs", name=f"xs{b}")
                base = xpad[:, :]
                win = bass.AP(base.tensor, base.offset + b * XPW, [[1, TAPS], [1, TP]])
                nc.scalar.dma_start(xs[1 : 1 + TAPS, :], win)
                # permuted columns: xs_p[:, 128j + f] (per q) = xs[:, 4f + j]
                xs_p = xsp.tile([KX, NQ, 4, 128], BF16, tag="xsp", name=f"xsp{b}")
                xs_ap = xs[:]
                pstep = xs_ap.ap[0][0]
                for q in range(NQ):
                    # rows 0..101 from base partition 0 (row 0 is junk and is
                    # overwritten with ones below)
                    pv = bass.AP(
                        xs_ap.tensor,
                        xs_ap.offset + q * 512,
                        [[pstep, KX], [1, 4], [4, 128]],
                    )
                    nc.gpsimd.tensor_copy(xs_p[:, q, :, :], pv)
                nc.vector.memset(xs_p[0:1, :, :, :], 1.0)

                qm_ps = psC.tile([1, A], FP32, tag="small", name=f"qm{b}")
                for ec in range(NE):
                    nc.tensor.matmul(
                        qm_ps[:],
                        lhsT=vdT[ec][:, 8 + b : 9 + b],
                        rhs=wdec_sb[ec][:],
                        start=(ec == 0),
                        stop=(ec == NE - 1),
                    )
                mext = mxp.tile([KX, A], BF16, tag="mext", name=f"mext{b}")
                nc.vector.tensor_copy(mext[:], m_base[:])
                nc.vector.scalar_tensor_tensor(
                    out=mext[0:1, :],
                    in0=qm_ps[:],
                    scalar=1.0,
                    in1=bconst[:],
                    op0=ALU.mult,
                    op1=ALU.add,
                )
                st["xs_p"] = xs_p
                st["mext"] = mext

            def emit_main(b, st):
                """keys+conv+bias matmuls, tanh, energy, exp(+partial sums)."""
                xqs, xs_p, mext = st["xqs"], st["xs_p"], st["mext"]
                exp_b = rows.tile([1, TP], BF16, tag="exp", bufs=2)
                # only the t>=2000 slots (q3, f>=116) need zeroing; they feed
                # the context broadcast where xt is already zero, but must
                # not be NaN
                expv = exp_b[0:1, :].rearrange("o (q j f) -> o q j f", q=4, j=4)
                nc.vector.memset(expv[:, 3, :, 116:128], 0.0)
                st["expv"] = expv
                zparts = rows.tile([1, 4], FP32, tag="zp")
                st["exp"] = exp_b
                st["zp"] = zparts
                for q in range(NQ):
                    t0 = q * 512
                    ths = []
                    for ac in range(NA):
                        ps = psA.tile([128, 512], FP32, tag="main")
                        for ec in range(NE):
                            nc.tensor.matmul(
                                ps[:],
                                lhsT=wenc_sb[ec][:, ac * 128 : (ac + 1) * 128],
                                rhs=xqs[q][:, :, ec, :],
                                start=(ec == 0),
                                stop=False,
                            )
                        nc.tensor.matmul(
                            ps[:],
                            lhsT=mext[:, ac * 128 : (ac + 1) * 128],
                            rhs=xs_p[:, q, :, :],
                            start=False,
                            stop=True,
                        )
                        th = thp.tile([128, 512], BF16, tag="tanh")
                        nc.scalar.activation(th[:], ps[:], AF.Tanh)
                        ths.append(th)
                    en_ps = psC.tile([1, 512], FP32, tag="small")
                    for ac in range(NA):
                        nc.tensor.matmul(
                            en_ps[:],
                            lhsT=vdT[ac][:, 0:1],
                            rhs=ths[ac][:],
                            start=(ac == 0),
                            stop=(ac == NA - 1),
                        )
                    if q < 3:
                        nc.scalar.activation(
                            exp_b[0:1, t0 : t0 + 512],
                            en_ps[:],
                            AF.Exp,
                            accum_out=zparts[0:1, q : q + 1],
                        )
                    else:
                        env = en_ps[0:1, :].rearrange("o (j f) -> o j f", j=4)
                        nc.scalar.activation(
                            expv[:, 3, :, 0:116],
                            env[:, :, 0:116],
                            AF.Exp,
                            accum_out=zparts[0:1, q : q + 1],
                        )

            def emit_softmax(b, st):
                exp_b, zparts = st["exp"], st["zp"]
                zsum = rows.tile([1, 1], FP32, tag="zs")
                nc.vector.tensor_reduce(zsum[:], zparts[:], AX.X, ALU.add)
                recip = rows.tile([1, 1], FP32, tag="rc")
                nc.vector.reciprocal(recip[:], zsum[:])
                expv = st["expv"]
                for q in range(NQ):
                    t0 = q * 512
                    if q < 3:
                        sl = exp_b[0:1, t0 : t0 + 512]
                    else:
                        sl = expv[:, 3, :, 0:116]
                    nc.scalar.activation(sl, sl, AF.Copy, scale=recip[:])
                # un-permute to t order (+ cast to f32) and write out
                attn32 = rows.tile([1, TP], FP32, tag="attn32", bufs=2)
                perm_in = exp_b[0:1, :].rearrange("o (q j f) -> o q f j", q=4, j=4)
                nc.vector.tensor_copy(
                    attn32[:].rearrange("o (q f j) -> o q f j", q=4, j=4), perm_in
                )
                nc.scalar.dma_start(out_attn[b : b + 1, :], attn32[0:1, 0:T])

            def emit_context(b, st):
                xqs, exp_b = st["xqs"], st["exp"]
                ctx_acc = ctxp.tile([128, 16], FP32, tag="ctxacc")
                for q in range(NQ):
                    t0 = q * 512
                    bc_ps = psB.tile([128, 512], FP32, tag="bcast")
                    nc.tensor.matmul(
                        bc_ps[:],
                        lhsT=ones_bf[:],
                        rhs=exp_b[0:1, t0 : t0 + 512],
                        start=True,
                        stop=True,
                    )
                    for ec in range(NE):
                        nc.vector.scalar_tensor_tensor(
                            out=junk[:],
                            in0=xqs[q][:, :, ec, :],
                            scalar=1.0,
                            in1=bc_ps[:],
                            op0=ALU.mult,
                            op1=ALU.mult,
                            accum_out=ctx_acc[:, ec * 4 + q : ec * 4 + q + 1],
                        )
                nc.vector.tensor_reduce(
                    ctx_all[:, b * 4 : (b + 1) * 4],
                    ctx_acc[:].rearrange("p (a g) -> p a g", g=4),
                    AX.X,
                    ALU.add,
                )

            # pipeline: prefetch loads one batch ahead; softmax+context of
            # batch b run right after main(b) so PE's bcast matmuls are not
            # queued behind main(b+1), and DVE context work overlaps it.
            states = {0: emit_enc_loads(0)}
            emit_w_loads()
            emit_aux_loads(0, states[0])
            states[1] = emit_enc_loads(1)
            emit_aux_loads(1, states[1])
            for b in range(BL):
                if b + 2 < BL:
                    states[b + 2] = emit_enc_loads(b + 2)
                    emit_aux_loads(b + 2, states[b + 2])
                emit_main(b, states[b])
                emit_softmax(b, states[b])
                emit_context(b, states[b])

            # context out: transpose [128 e, 16 (b,ec)] -> [16, 128]
            ctxT = wts.tile([32, 128], FP32, tag="ctxT")
            for bi in range(4):
                nc.vector.transpose(
                    ctxT[0:32, 32 * bi : 32 * (bi + 1)],
                    ctx_all[32 * bi : 32 * (bi + 1), 0:32],
                )
            nc.scalar.dma_start(out_ctx[:, :], ctxT[0 : BL * NE, :])

    nc.compile()
    return nc


_NC = None


def _get_nc():
    global _NC
    if _NC is None:
        _NC = build_nc()
    return _NC


def kernel(**inputs):
    nc = _get_nc()
    full = {k: np.ascontiguousarray(np.asarray(v, np.float32)) for k, v in inputs.items()}
    in_maps = []
    for i in range(NCORES):
        sl = slice(i * BL, (i + 1) * BL)
        m = {
            "encoder_states": full["encoder_states"][sl],
            "decoder_outputs": full["decoder_outputs"][sl],
            "attention_weights_step": full["attention_weights_step"][sl],
        }
        for k in ("W_enc", "b_enc", "W_dec", "b_dec", "conv_w", "conv_b", "W_fil", "b_fil", "v_a"):
            m[k] = full[k]
        in_maps.append(m)
    res = run_bass_kernel_spmd(nc, in_maps, core_ids=list(range(NCORES)))
    attn = np.concatenate([r["out_attn"] for r in res.results], axis=0)
    ctx = np.concatenate(
        [r["out_ctx"].reshape(BL, E)[:, None, :] for r in res.results], axis=0
    )
    return (ctx.astype(np.float32), attn.astype(np.float32))


# revision 23
# speedup vs baseline: 1.0930x; 1.0258x over previous
"""Trainium2 Bass kernel for hybrid location-sensitive attention.

Problem: nn_AttentionMechanism_54752243089428
  keys  = enc @ W_enc + b_enc                       [B,T,A]
  query = dec @ W_dec + b_dec                       [B,1,A]
  conv  = Conv1d(attn_prev, conv_w) + conv_b        [B,10,T]
  cfeat = conv^T @ W_fil + b_fil                    [B,T,A]
  energy= tanh(keys+query+cfeat) @ v_a (+b_va)      [B,T]
  attn  = softmax(energy, axis=T)
  ctx   = sum_t attn * enc                          [B,1,E]
  returns (ctx, attn)

Strategy (8 NeuronCores, data-parallel over batch, 4 batches/core):
 - All big compute in bf16 (rel-err gate is 2e-2; bf16 keeps ~0.5%).
 - Layout: [A/E on partitions, T on free].  enc is cast f32->bf16 during
   DMA (SWDGE) into [t,e] staging tiles, then one xbar DMA-transpose per
   512-t quarter produces xt tiles laid out [p, j, ec, f] (e=128*ec+p,
   t=512*q+128*j+f).  All transposes issue from the Sync engine only
   (concurrent xbar use from two HWDGE engines races on xbar state).
 - Conv is folded into the keys matmul: conv_feat^T = M^T xs where
   M[k,a] = sum_c conv_w[c,k] W_fil[c,a] (precomputed on PE) and
   xs[k,t] = xpad[t+k] is a shifted-window matrix built by one strided
   DMA from a padded copy of attn_prev in scratch DRAM.  Row 0 is ones,
   paired with a per-batch row qb = dec@W_dec + (all biases), so one
   PSUM accumulation group produces keys+query+conv_feat+biases.
 - tanh on ScalarE (PSUM->SBUF), energy = v^T tanh via M=1 matmuls,
   softmax without max-subtraction (|energy| <= sum|v| ~ 11, exp safe),
   exp+sum fused via activation accum_out.  t is processed in four
   512 chunks; t in [2000, 2048) is zero-padded and masked out of exp.
 - context via scalar_tensor_tensor (fused multiply + free-dim reduce)
   on VectorE against a PE-broadcast normalized-attn row.  b_va shifts
   all energies equally -> cancels in softmax -> ignored.
"""

import os
import sys

sys.path.insert(0, "/opt/trn_rl_repo")

KVAR = os.environ.get("KVAR", "full")

import numpy as np

import concourse.bass as bass
import concourse.mybir as mybir
import concourse.tile as tile
from concourse import bacc
from concourse.bass_utils import run_bass_kernel_spmd

FP32 = mybir.dt.float32
BF16 = mybir.dt.bfloat16
ALU = mybir.AluOpType
AF = mybir.ActivationFunctionType
AX = mybir.AxisListType

B, T, E, A = 32, 2000, 512, 512
OUT_CH, TAPS, PADK = 10, 101, 50
NCORES = 8
BL = B // NCORES  # 4 batches per core
KX = TAPS + 1  # ones row + taps
TP = 2048  # padded T (t in [2000,2048) zeroed/masked)
XPW = 2176  # padded attn_prev row width (>= TP + PADK + TAPS)
NE = E // 128  # 4 e-chunks
NA = A // 128  # 4 a-chunks
NQ = 4  # t quarters of 512
TVAL = [512, 512, 512, 464]  # valid t per quarter


def build_nc():
    nc = bacc.Bacc("TRN2", target_bir_lowering=False)

    enc = nc.declare_dram_parameter("encoder_states", [BL, T, E], FP32, isOutput=False)
    dec = nc.declare_dram_parameter("decoder_outputs", [BL, 1, E], FP32, isOutput=False)
    awt = nc.declare_dram_parameter("attention_weights_step", [BL, T], FP32, isOutput=False)
    w_enc = nc.declare_dram_parameter("W_enc", [E, A], FP32, isOutput=False)
    b_enc = nc.declare_dram_parameter("b_enc", [A], FP32, isOutput=False)
    w_dec = nc.declare_dram_parameter("W_dec", [E, A], FP32, isOutput=False)
    b_dec = nc.declare_dram_parameter("b_dec", [A], FP32, isOutput=False)
    conv_w = nc.declare_dram_parameter("conv_w", [OUT_CH, 1, TAPS], FP32, isOutput=False)
    conv_b = nc.declare_dram_parameter("conv_b", [OUT_CH], FP32, isOutput=False)
    w_fil = nc.declare_dram_parameter("W_fil", [OUT_CH, A], FP32, isOutput=False)
    b_fil = nc.declare_dram_parameter("b_fil", [A], FP32, isOutput=False)
    v_a = nc.declare_dram_parameter("v_a", [A], FP32, isOutput=False)

    out_attn = nc.declare_dram_parameter("out_attn", [BL, T], FP32, isOutput=True)
    out_ctx = nc.declare_dram_parameter("out_ctx", [BL * NE, 128], FP32, isOutput=True)

    with tile.TileContext(nc) as tc:
        with (
            tc.tile_pool(name="wts", bufs=1) as wts,
            tc.tile_pool(name="xt", bufs=16) as xtp,
            tc.tile_pool(name="stg", bufs=6) as stgp,
            tc.tile_pool(name="wf", bufs=2) as wfp,
            tc.tile_pool(name="xs", bufs=4) as xsp,
            tc.tile_pool(name="mext", bufs=4) as mxp,
            tc.tile_pool(name="tanh", bufs=8) as thp,
            tc.tile_pool(name="rows", bufs=3) as rows,
            tc.tile_pool(name="ctx", bufs=2) as ctxp,
            tc.tile_pool(name="psA", bufs=3, space="PSUM") as psA,
            tc.tile_pool(name="psB", bufs=3, space="PSUM") as psB,
            tc.tile_pool(name="psC", bufs=2, space="PSUM") as psC,
            tc.tile_pool(name="dram", bufs=1, space="DRAM") as dramp,
        ):
            # ---------------- weights / constants ----------------
            # small loads first so the critical-path chain (vd32 -> vdT ->
            # qm -> mext, M, biases) is not queued behind megabyte loads
            wfil_sb = wts.tile([OUT_CH, A], BF16, tag="wfil")
            nc.gpsimd.dma_start(wfil_sb[:], w_fil[:, :])
            cw_sb = wts.tile([OUT_CH, KX], BF16, tag="cw")
            nc.vector.memset(cw_sb[:], 0.0)
            nc.gpsimd.dma_start(cw_sb[:, 1 : 1 + TAPS], conv_w[:, 0, :])
            cb_sb = wts.tile([OUT_CH, 1], BF16, tag="cb")
            nc.gpsimd.dma_start(cb_sb[:], conv_b.rearrange("(p o) -> p o", o=1))

            ber = wts.tile([1, A], FP32, tag="ber")
            nc.scalar.dma_start(ber[:], b_enc[None, :])
            bdr = wts.tile([1, A], FP32, tag="bdr")
            nc.scalar.dma_start(bdr[:], b_dec[None, :])
            bfr = wts.tile([1, A], FP32, tag="bfr")
            nc.scalar.dma_start(bfr[:], b_fil[None, :])

            # v_a and decoder outputs, transposed to column layout via DVE
            vd32 = wts.tile([32, E], BF16, tag="vd32")
            nc.vector.memset(vd32[:], 0.0)
            nc.gpsimd.dma_start(vd32[0:1, :], v_a[None, :])
            nc.gpsimd.dma_start(vd32[8 : 8 + BL, :], dec[:, 0, :])

            wenc_sb = []
            wdec_sb = []

            def emit_w_loads():
                for ec in range(NE):
                    wf = wfp.tile([128, A], FP32, tag="wf32", name=f"wef{ec}")
                    nc.scalar.dma_start(wf[:], w_enc[ec * 128 : (ec + 1) * 128, :])
                    t_we = wts.tile([128, A], BF16, tag=f"wenc{ec}", name=f"we{ec}")
                    nc.gpsimd.tensor_copy(t_we[:], wf[:])
                    wenc_sb.append(t_we)
                for ec in range(NE):
                    wf = wfp.tile([128, A], FP32, tag="wf32", name=f"wdf{ec}")
                    nc.scalar.dma_start(wf[:], w_dec[ec * 128 : (ec + 1) * 128, :])
                    t_wd = wts.tile([128, A], BF16, tag=f"wdec{ec}", name=f"wd{ec}")
                    nc.gpsimd.tensor_copy(t_wd[:], wf[:])
                    wdec_sb.append(t_wd)
            vdT = []
            for ec in range(NE):
                t_vdT = wts.tile([128, 32], BF16, tag=f"vdT{ec}")
                for bi in range(4):
                    nc.vector.transpose(
                        t_vdT[32 * bi : 32 * (bi + 1), :],
                        vd32[:, ec * 128 + 32 * bi : ec * 128 + 32 * (bi + 1)],
                    )
                vdT.append(t_vdT)

            ones_bf = wts.tile([1, 128], BF16, tag="ones")
            nc.vector.memset(ones_bf[:], 1.0)

            # M = conv_w^T @ W_fil (row 0 zero); cbW = conv_b @ W_fil
            m_ps = psA.tile([KX, A], FP32, tag="main")
            nc.tensor.matmul(m_ps[:], lhsT=cw_sb[:], rhs=wfil_sb[:], start=True, stop=True)
            m_base = wts.tile([KX, A], BF16, tag="mbase")
            nc.scalar.copy(m_base[:], m_ps[:])
            cbw_ps = psC.tile([1, A], FP32, tag="small")
            nc.tensor.matmul(cbw_ps[:], lhsT=cb_sb[:], rhs=wfil_sb[:], start=True, stop=True)
            bconst = wts.tile([1, A], FP32, tag="bconst")
            nc.vector.tensor_add(bconst[:], ber[:], bdr[:])
            nc.vector.tensor_add(bconst[:], bconst[:], bfr[:])
            nc.vector.tensor_add(bconst[:], bconst[:], cbw_ps[:])

            # ---------------- padded attn_prev in scratch DRAM ----------------
            xpad = dramp.tile([BL, XPW], BF16)
            zedge = wts.tile([BL, 128], BF16, tag="zedge")
            nc.vector.memset(zedge[:], 0.0)
            nc.scalar.dma_start(xpad[:, 0:PADK], zedge[:, 0:PADK])
            nc.scalar.dma_start(xpad[:, PADK + T : XPW], zedge[:, 0 : XPW - PADK - T])

            def emit_xpad_fill():
                nc.gpsimd.dma_start(xpad[:, PADK : PADK + T], awt[:, :])

            junk = wts.tile([128, 512], BF16, tag="junk")
            ctx_all = wts.tile([128, 32], FP32, tag="ctxall")
            nc.vector.memset(ctx_all[:], 0.0)

            # ---------------- per-batch stages ----------------
            def emit_enc_loads(b):
                """enc -> bf16 staging (partition p holds 4 consecutive t
                rows -> one 8 KB contiguous read per partition) -> one xbar
                transpose per 512-t quarter.  Resulting xt column order
                within a quarter is t = 4f + j (consistently permuted)."""
                xqs = []
                for q in range(NQ):
                    xq = xtp.tile([128, 4, NE, 128], BF16, tag="xt", name=f"xt_b{b}q{q}")
                    xqs.append(xq)
                for q in range(3):
                    stg = stgp.tile([128, 2048], BF16, tag="stg")
                    nc.gpsimd.dma_start(
                        stg[:],
                        enc[b, q * 512 : (q + 1) * 512, :].rearrange(
                            "(p j) e -> p (j e)", j=4
                        ),
                    )
                    nc.sync.dma_start(
                        xqs[q][:].rearrange("p j c f -> p (j c) f"),
                        stg[:],
                        transpose=True,
                    )
                # quarter 3: valid t rows 1536..2000 = 116 partitions * 4;
                # zero partitions 116.. so t in [2000,2048) transposes to 0
                stg3 = stgp.tile([128, 2048], BF16, tag="stg")
                # 32-aligned partition base; rows 96:116 are overwritten by
                # the load below, rows 116:128 stay zero
                nc.vector.memset(stg3[96:128, :], 0.0)
                nc.gpsimd.dma_start(
                    stg3[0:116, :],
                    enc[b, 1536:2000, :].rearrange("(p j) e -> p (j e)", j=4),
                )
                nc.sync.dma_start(
                    xqs[3][:].rearrange("p j c f -> p (j c) f"),
                    stg3[:],
                    transpose=True,
                )
                return {"xqs": xqs}

            def emit_aux_loads(b, st):
                """shifted-window matrix (+ column-permuted copy) and the
                per-batch bias row."""
                xs = xsp.tile([KX, TP], BF16, tag="xs", name=f"xs{b}", bufs=2)
                base = xpad[:, :]
                win = bass.AP(base.tensor, base.offset + b * XPW, [[1, TAPS], [1, TP]])
                nc.scalar.dma_start(xs[1 : 1 + TAPS, :], win)
                # permuted columns: xs_p[:, 128j + f] (per q) = xs[:, 4f + j]
                xs_p = xsp.tile([KX, NQ, 4, 128], BF16, tag="xsp", name=f"xsp{b}")
                xs_ap = xs[:]
                pstep = xs_ap.ap[0][0]
                for q in range(NQ):
                    # rows 0..101 from base partition 0 (row 0 is junk and is
                    # overwritten with ones below)
                    pv = bass.AP(
                        xs_ap.tensor,
                        xs_ap.offset + q * 512,
                        [[pstep, KX], [1, 4], [4, 128]],
                    )
                    nc.gpsimd.tensor_copy(xs_p[:, q, :, :], pv)
                nc.vector.memset(xs_p[0:1, :, :, :], 1.0)

                qm_ps = psC.tile([1, A], FP32, tag="small", name=f"qm{b}")
                for ec in range(NE):
                    nc.tensor.matmul(
                        qm_ps[:],
                        lhsT=vdT[ec][:, 8 + b : 9 + b],
                        rhs=wdec_sb[ec][:],
                        start=(ec == 0),
                        stop=(ec == NE - 1),
                    )
                mext = mxp.tile([KX, A], BF16, tag="mext", name=f"mext{b}")
                nc.vector.tensor_copy(mext[:], m_base[:])
                nc.vector.scalar_tensor_tensor(
                    out=mext[0:1, :],
                    in0=qm_ps[:],
                    scalar=1.0,
                    in1=bconst[:],
                    op0=ALU.mult,
                    op1=ALU.add,
                )
                st["xs_p"] = xs_p
                st["mext"] = mext

            def emit_main(b, st):
                """keys+conv+bias matmuls, tanh, energy, exp(+partial sums)."""
                xqs, xs_p, mext = st["xqs"], st["xs_p"], st["mext"]
                exp_b = rows.tile([1, TP], BF16, tag="exp", bufs=2)
                # only the t>=2000 slots (q3, f>=116) need zeroing; they feed
                # the context broadcast where xt is already zero, but must
                # not be NaN
                expv = exp_b[0:1, :].rearrange("o (q j f) -> o q j f", q=4, j=4)
                nc.vector.memset(expv[:, 3, :, 116:128], 0.0)
                st["expv"] = expv
                zparts = rows.tile([1, 4], FP32, tag="zp")
                st["exp"] = exp_b
                st["zp"] = zparts
                for q in range(NQ):
                    t0 = q * 512
                    ths = []
                    for ac in range(NA):
                        ps = psA.tile([128, 512], FP32, tag="main")
                        for ec in range(NE):
                            nc.tensor.matmul(
                                ps[:],
                                lhsT=wenc_sb[ec][:, ac * 128 : (ac + 1) * 128],
                                rhs=xqs[q][:, :, ec, :],
                                start=(ec == 0),
                                stop=False,
                            )
                        nc.tensor.matmul(
                            ps[:],
                            lhsT=mext[:, ac * 128 : (ac + 1) * 128],
                            rhs=xs_p[:, q, :, :],
                            start=False,
                            stop=True,
                        )
                        th = thp.tile([128, 512], BF16, tag="tanh")
                        nc.scalar.activation(th[:], ps[:], AF.Tanh)
                        ths.append(th)
                    en_ps = psC.tile([1, 512], FP32, tag="small")
                    for ac in range(NA):
                        nc.tensor.matmul(
                            en_ps[:],
                            lhsT=vdT[ac][:, 0:1],
                            rhs=ths[ac][:],
                            start=(ac == 0),
                            stop=(ac == NA - 1),
                        )
                    if q < 3:
                        nc.scalar.activation(
                            exp_b[0:1, t0 : t0 + 512],
                            en_ps[:],
                            AF.Exp,
                            accum_out=zparts[0:1, q : q + 1],
                        )
                    else:
                        env = en_ps[0:1, :].rearrange("o (j f) -> o j f", j=4)
                        nc.scalar.activation(
                            expv[:, 3, :, 0:116],
                            env[:, :, 0:116],
                            AF.Exp,
                            accum_out=zparts[0:1, q : q + 1],
                        )

            def emit_softmax(b, st):
                exp_b, zparts = st["exp"], st["zp"]
                zsum = rows.tile([1, 1], FP32, tag="zs")
                nc.vector.tensor_reduce(zsum[:], zparts[:], AX.X, ALU.add)
                recip = rows.tile([1, 1], FP32, tag="rc")
                nc.vector.reciprocal(recip[:], zsum[:])
                expv = st["expv"]
                for q in range(NQ):
                    t0 = q * 512
                    if q < 3:
                        sl = exp_b[0:1, t0 : t0 + 512]
                    else:
                        sl = expv[:, 3, :, 0:116]
                    nc.scalar.activation(sl, sl, AF.Copy, scale=recip[:])
                # un-permute to t order (+ cast to f32) and write out
                attn32 = rows.tile([1, TP], FP32, tag="attn32", bufs=2)
                perm_in = exp_b[0:1, :].rearrange("o (q j f) -> o q f j", q=4, j=4)
                nc.vector.tensor_copy(
                    attn32[:].rearrange("o (q f j) -> o q f j", q=4, j=4), perm_in
                )
                nc.scalar.dma_start(out_attn[b : b + 1, :], attn32[0:1, 0:T])

            def emit_context(b, st):
                xqs, exp_b = st["xqs"], st["exp"]
                ctx_acc = ctxp.tile([128, 16], FP32, tag="ctxacc")
                for q in range(NQ):
                    t0 = q * 512
                    bc_ps = psB.tile([128, 512], FP32, tag="bcast")
                    nc.tensor.matmul(
                        bc_ps[:],
                        lhsT=ones_bf[:],
                        rhs=exp_b[0:1, t0 : t0 + 512],
                        start=True,
                        stop=True,
                    )
                    for ec in range(NE):
                        nc.vector.scalar_tensor_tensor(
                            out=junk[:],
                            in0=xqs[q][:, :, ec, :],
                            scalar=1.0,
                            in1=bc_ps[:],
                            op0=ALU.mult,
                            op1=ALU.mult,
                            accum_out=ctx_acc[:, ec * 4 + q : ec * 4 + q + 1],
                        )
                nc.vector.tensor_reduce(
                    ctx_all[:, b * 4 : (b + 1) * 4],
                    ctx_acc[:].rearrange("p (a g) -> p a g", g=4),
                    AX.X,
                    ALU.add,
                )

            # pipeline: prefetch loads one batch ahead; softmax+context of
            # batch b run right after main(b) so PE's bcast matmuls are not
            # queued behind main(b+1), and DVE context work overlaps it.
            states = {0: emit_enc_loads(0)}
            emit_xpad_fill()
            emit_w_loads()
            emit_aux_loads(0, states[0])
            states[1] = emit_enc_loads(1)
            emit_aux_loads(1, states[1])
            for b in range(BL):
                if b + 2 < BL:
                    states[b + 2] = emit_enc_loads(b + 2)
                    emit_aux_loads(b + 2, states[b + 2])
                emit_main(b, states[b])
                emit_softmax(b, states[b])
                emit_context(b, states[b])

            # context out: transpose [128 e, 16 (b,ec)] -> [16, 128]
            ctxT = wts.tile([32, 128], FP32, tag="ctxT")
            for bi in range(4):
                nc.vector.transpose(
                    ctxT[0:32, 32 * bi : 32 * (bi + 1)],
                    ctx_all[32 * bi : 32 * (bi + 1), 0:32],
                )
            nc.scalar.dma_start(out_ctx[:, :], ctxT[0 : BL * NE, :])

    nc.compile()
    return nc


_NC = None


def _get_nc():
    global _NC
    if _NC is None:
        _NC = build_nc()
    return _NC


def kernel(**inputs):
    nc = _get_nc()
    full = {k: np.ascontiguousarray(np.asarray(v, np.float32)) for k, v in inputs.items()}
    in_maps = []
    for i in range(NCORES):
        sl = slice(i * BL, (i + 1) * BL)
        m = {
            "encoder_states": full["encoder_states"][sl],
            "decoder_outputs": full["decoder_outputs"][sl],
            "attention_weights_step": full["attention_weights_step"][sl],
        }
        for k in ("W_enc", "b_enc", "W_dec", "b_dec", "conv_w", "conv_b", "W_fil", "b_fil", "v_a"):
            m[k] = full[k]
        in_maps.append(m)
    res = run_bass_kernel_spmd(nc, in_maps, core_ids=list(range(NCORES)))
    attn = np.concatenate([r["out_attn"] for r in res.results], axis=0)
    ctx = np.concatenate(
        [r["out_ctx"].reshape(BL, E)[:, None, :] for r in res.results], axis=0
    )
    return (ctx.astype(np.float32), attn.astype(np.float32))


# revision 24
# speedup vs baseline: 1.0937x; 1.0007x over previous
"""Trainium2 Bass kernel for hybrid location-sensitive attention.

Problem: nn_AttentionMechanism_54752243089428
  keys  = enc @ W_enc + b_enc                       [B,T,A]
  query = dec @ W_dec + b_dec                       [B,1,A]
  conv  = Conv1d(attn_prev, conv_w) + conv_b        [B,10,T]
  cfeat = conv^T @ W_fil + b_fil                    [B,T,A]
  energy= tanh(keys+query+cfeat) @ v_a (+b_va)      [B,T]
  attn  = softmax(energy, axis=T)
  ctx   = sum_t attn * enc                          [B,1,E]
  returns (ctx, attn)

Strategy (8 NeuronCores, data-parallel over batch, 4 batches/core):
 - All big compute in bf16 (rel-err gate is 2e-2; bf16 keeps ~0.5%).
 - Layout: [A/E on partitions, T on free].  enc is cast f32->bf16 during
   DMA (SWDGE) into [t,e] staging tiles, then one xbar DMA-transpose per
   512-t quarter produces xt tiles laid out [p, j, ec, f] (e=128*ec+p,
   t=512*q+128*j+f).  All transposes issue from the Sync engine only
   (concurrent xbar use from two HWDGE engines races on xbar state).
 - Conv is folded into the keys matmul: conv_feat^T = M^T xs where
   M[k,a] = sum_c conv_w[c,k] W_fil[c,a] (precomputed on PE) and
   xs[k,t] = xpad[t+k] is a shifted-window matrix built by one strided
   DMA from a padded copy of attn_prev in scratch DRAM.  Row 0 is ones,
   paired with a per-batch row qb = dec@W_dec + (all biases), so one
   PSUM accumulation group produces keys+query+conv_feat+biases.
 - tanh on ScalarE (PSUM->SBUF), energy = v^T tanh via M=1 matmuls,
   softmax without max-subtraction (|energy| <= sum|v| ~ 11, exp safe),
   exp+sum fused via activation accum_out.  t is processed in four
   512 chunks; t in [2000, 2048) is zero-padded and masked out of exp.
 - context via scalar_tensor_tensor (fused multiply + free-dim reduce)
   on VectorE against a PE-broadcast normalized-attn row.  b_va shifts
   all energies equally -> cancels in softmax -> ignored.
"""

import os
import sys

sys.path.insert(0, "/opt/trn_rl_repo")

KVAR = os.environ.get("KVAR", "full")

import numpy as np

import concourse.bass as bass
import concourse.mybir as mybir
import concourse.tile as tile
from concourse import bacc
from concourse.bass_utils import run_bass_kernel_spmd

FP32 = mybir.dt.float32
BF16 = mybir.dt.bfloat16
ALU = mybir.AluOpType
AF = mybir.ActivationFunctionType
AX = mybir.AxisListType

B, T, E, A = 32, 2000, 512, 512
OUT_CH, TAPS, PADK = 10, 101, 50
NCORES = 8
BL = B // NCORES  # 4 batches per core
KX = TAPS + 1  # ones row + taps
TP = 2048  # padded T (t in [2000,2048) zeroed/masked)
XPW = 2176  # padded attn_prev row width (>= TP + PADK + TAPS)
NE = E // 128  # 4 e-chunks
NA = A // 128  # 4 a-chunks
NQ = 4  # t quarters of 512
TVAL = [512, 512, 512, 464]  # valid t per quarter


def build_nc():
    nc = bacc.Bacc("TRN2", target_bir_lowering=False)

    enc = nc.declare_dram_parameter("encoder_states", [BL, T, E], FP32, isOutput=False)
    dec = nc.declare_dram_parameter("decoder_outputs", [BL, 1, E], FP32, isOutput=False)
    awt = nc.declare_dram_parameter("attention_weights_step", [BL, T], FP32, isOutput=False)
    w_enc = nc.declare_dram_parameter("W_enc", [E, A], FP32, isOutput=False)
    b_enc = nc.declare_dram_parameter("b_enc", [A], FP32, isOutput=False)
    w_dec = nc.declare_dram_parameter("W_dec", [E, A], FP32, isOutput=False)
    b_dec = nc.declare_dram_parameter("b_dec", [A], FP32, isOutput=False)
    conv_w = nc.declare_dram_parameter("conv_w", [OUT_CH, 1, TAPS], FP32, isOutput=False)
    conv_b = nc.declare_dram_parameter("conv_b", [OUT_CH], FP32, isOutput=False)
    w_fil = nc.declare_dram_parameter("W_fil", [OUT_CH, A], FP32, isOutput=False)
    b_fil = nc.declare_dram_parameter("b_fil", [A], FP32, isOutput=False)
    v_a = nc.declare_dram_parameter("v_a", [A], FP32, isOutput=False)

    out_attn = nc.declare_dram_parameter("out_attn", [BL, T], FP32, isOutput=True)
    out_ctx = nc.declare_dram_parameter("out_ctx", [BL * NE, 128], FP32, isOutput=True)

    with tile.TileContext(nc) as tc:
        with (
            tc.tile_pool(name="wts", bufs=1) as wts,
            tc.tile_pool(name="xt", bufs=16) as xtp,
            tc.tile_pool(name="stg", bufs=8) as stgp,
            tc.tile_pool(name="wf", bufs=2) as wfp,
            tc.tile_pool(name="xs", bufs=4) as xsp,
            tc.tile_pool(name="mext", bufs=4) as mxp,
            tc.tile_pool(name="tanh", bufs=8) as thp,
            tc.tile_pool(name="rows", bufs=3) as rows,
            tc.tile_pool(name="ctx", bufs=2) as ctxp,
            tc.tile_pool(name="psA", bufs=3, space="PSUM") as psA,
            tc.tile_pool(name="psB", bufs=3, space="PSUM") as psB,
            tc.tile_pool(name="psC", bufs=2, space="PSUM") as psC,
            tc.tile_pool(name="dram", bufs=1, space="DRAM") as dramp,
        ):
            # ---------------- weights / constants ----------------
            # small loads first so the critical-path chain (vd32 -> vdT ->
            # qm -> mext, M, biases) is not queued behind megabyte loads
            wfil_sb = wts.tile([OUT_CH, A], BF16, tag="wfil")
            nc.gpsimd.dma_start(wfil_sb[:], w_fil[:, :])
            cw_sb = wts.tile([OUT_CH, KX], BF16, tag="cw")
            nc.vector.memset(cw_sb[:], 0.0)
            nc.gpsimd.dma_start(cw_sb[:, 1 : 1 + TAPS], conv_w[:, 0, :])
            cb_sb = wts.tile([OUT_CH, 1], BF16, tag="cb")
            nc.gpsimd.dma_start(cb_sb[:], conv_b.rearrange("(p o) -> p o", o=1))

            ber = wts.tile([1, A], FP32, tag="ber")
            nc.scalar.dma_start(ber[:], b_enc[None, :])
            bdr = wts.tile([1, A], FP32, tag="bdr")
            nc.scalar.dma_start(bdr[:], b_dec[None, :])
            bfr = wts.tile([1, A], FP32, tag="bfr")
            nc.scalar.dma_start(bfr[:], b_fil[None, :])

            # v_a and decoder outputs, transposed to column layout via DVE
            vd32 = wts.tile([32, E], BF16, tag="vd32")
            nc.vector.memset(vd32[:], 0.0)
            nc.gpsimd.dma_start(vd32[0:1, :], v_a[None, :])
            nc.gpsimd.dma_start(vd32[8 : 8 + BL, :], dec[:, 0, :])

            wenc_sb = []
            wdec_sb = []

            def emit_w_loads():
                for ec in range(NE):
                    wfd = wfp.tile([128, A], FP32, tag="wf32", name=f"wdf{ec}")
                    nc.scalar.dma_start(wfd[:], w_dec[ec * 128 : (ec + 1) * 128, :])
                    t_wd = wts.tile([128, A], BF16, tag=f"wdec{ec}", name=f"wd{ec}")
                    nc.scalar.copy(t_wd[:], wfd[:])
                    wdec_sb.append(t_wd)
                    wfe = wfp.tile([128, A], FP32, tag="wf32", name=f"wef{ec}")
                    nc.scalar.dma_start(wfe[:], w_enc[ec * 128 : (ec + 1) * 128, :])
                    t_we = wts.tile([128, A], BF16, tag=f"wenc{ec}", name=f"we{ec}")
                    nc.scalar.copy(t_we[:], wfe[:])
                    wenc_sb.append(t_we)

            vdT = []
            for ec in range(NE):
                t_vdT = wts.tile([128, 32], BF16, tag=f"vdT{ec}")
                for bi in range(4):
                    nc.vector.transpose(
                        t_vdT[32 * bi : 32 * (bi + 1), :],
                        vd32[:, ec * 128 + 32 * bi : ec * 128 + 32 * (bi + 1)],
                    )
                vdT.append(t_vdT)

            ones_bf = wts.tile([1, 128], BF16, tag="ones")
            nc.vector.memset(ones_bf[:], 1.0)

            # M = conv_w^T @ W_fil (row 0 zero); cbW = conv_b @ W_fil
            m_ps = psA.tile([KX, A], FP32, tag="main")
            nc.tensor.matmul(m_ps[:], lhsT=cw_sb[:], rhs=wfil_sb[:], start=True, stop=True)
            m_base = wts.tile([KX, A], BF16, tag="mbase")
            nc.scalar.copy(m_base[:], m_ps[:])
            cbw_ps = psC.tile([1, A], FP32, tag="small")
            nc.tensor.matmul(cbw_ps[:], lhsT=cb_sb[:], rhs=wfil_sb[:], start=True, stop=True)
            bconst = wts.tile([1, A], FP32, tag="bconst")
            nc.vector.tensor_add(bconst[:], ber[:], bdr[:])
            nc.vector.tensor_add(bconst[:], bconst[:], bfr[:])
            nc.vector.tensor_add(bconst[:], bconst[:], cbw_ps[:])

            # ---------------- padded attn_prev in scratch DRAM ----------------
            xpad = dramp.tile([BL, XPW], BF16)
            zedge = wts.tile([BL, 128], BF16, tag="zedge")
            nc.vector.memset(zedge[:], 0.0)
            nc.scalar.dma_start(xpad[:, 0:PADK], zedge[:, 0:PADK])
            nc.scalar.dma_start(xpad[:, PADK + T : XPW], zedge[:, 0 : XPW - PADK - T])

            def emit_xpad_fill():
                nc.gpsimd.dma_start(xpad[:, PADK : PADK + T], awt[:, :])

            junk = wts.tile([128, 512], BF16, tag="junk")
            ctx_all = wts.tile([128, 32], FP32, tag="ctxall")
            nc.vector.memset(ctx_all[:], 0.0)

            # ---------------- per-batch stages ----------------
            def emit_enc_loads(b):
                """enc -> bf16 staging (partition p holds 4 consecutive t
                rows -> one 8 KB contiguous read per partition) -> one xbar
                transpose per 512-t quarter.  Resulting xt column order
                within a quarter is t = 4f + j (consistently permuted)."""
                xqs = []
                for q in range(NQ):
                    xq = xtp.tile([128, 4, NE, 128], BF16, tag="xt", name=f"xt_b{b}q{q}")
                    xqs.append(xq)
                for q in range(3):
                    stg = stgp.tile([128, 2048], BF16, tag="stg")
                    nc.gpsimd.dma_start(
                        stg[:],
                        enc[b, q * 512 : (q + 1) * 512, :].rearrange(
                            "(p j) e -> p (j e)", j=4
                        ),
                    )
                    nc.sync.dma_start(
                        xqs[q][:].rearrange("p j c f -> p (j c) f"),
                        stg[:],
                        transpose=True,
                    )
                # quarter 3: valid t rows 1536..2000 = 116 partitions * 4;
                # zero partitions 116.. so t in [2000,2048) transposes to 0
                stg3 = stgp.tile([128, 2048], BF16, tag="stg")
                # 32-aligned partition base; rows 96:116 are overwritten by
                # the load below, rows 116:128 stay zero
                nc.vector.memset(stg3[96:128, :], 0.0)
                nc.gpsimd.dma_start(
                    stg3[0:116, :],
                    enc[b, 1536:2000, :].rearrange("(p j) e -> p (j e)", j=4),
                )
                nc.sync.dma_start(
                    xqs[3][:].rearrange("p j c f -> p (j c) f"),
                    stg3[:],
                    transpose=True,
                )
                return {"xqs": xqs}

            def emit_xs_loads(b, st):
                """shifted-window matrix + column-permuted copy (DVE)."""
                xs = xsp.tile([KX, TP], BF16, tag="xs", name=f"xs{b}", bufs=2)
                base = xpad[:, :]
                win = bass.AP(base.tensor, base.offset + b * XPW, [[1, TAPS], [1, TP]])
                nc.scalar.dma_start(xs[1 : 1 + TAPS, :], win)
                xs_p = xsp.tile([KX, NQ, 4, 128], BF16, tag="xsp", name=f"xsp{b}")
                xs_ap = xs[:]
                pstep = xs_ap.ap[0][0]
                for q in range(NQ):
                    pv = bass.AP(
                        xs_ap.tensor,
                        xs_ap.offset + q * 512,
                        [[pstep, KX], [1, 4], [4, 128]],
                    )
                    nc.vector.tensor_copy(xs_p[:, q, :, :], pv)
                nc.vector.memset(xs_p[0:1, :, :, :], 1.0)
                st["xs_p"] = xs_p

            def emit_qm(b, st):
                qm_ps = psC.tile([1, A], FP32, tag="small", name=f"qm{b}")
                for ec in range(NE):
                    nc.tensor.matmul(
                        qm_ps[:],
                        lhsT=vdT[ec][:, 8 + b : 9 + b],
                        rhs=wdec_sb[ec][:],
                        start=(ec == 0),
                        stop=(ec == NE - 1),
                    )
                mext = mxp.tile([KX, A], BF16, tag="mext", name=f"mext{b}")
                nc.vector.tensor_copy(mext[:], m_base[:])
                nc.vector.scalar_tensor_tensor(
                    out=mext[0:1, :],
                    in0=qm_ps[:],
                    scalar=1.0,
                    in1=bconst[:],
                    op0=ALU.mult,
                    op1=ALU.add,
                )
                st["mext"] = mext

            def emit_main(b, st):
                """keys+conv+bias matmuls, tanh, energy, exp(+partial sums)."""
                xqs, xs_p, mext = st["xqs"], st["xs_p"], st["mext"]
                exp_b = rows.tile([1, TP], BF16, tag="exp", bufs=2)
                # only the t>=2000 slots (q3, f>=116) need zeroing; they feed
                # the context broadcast where xt is already zero, but must
                # not be NaN
                expv = exp_b[0:1, :].rearrange("o (q j f) -> o q j f", q=4, j=4)
                nc.vector.memset(expv[:, 3, :, 116:128], 0.0)
                st["expv"] = expv
                zparts = rows.tile([1, 4], FP32, tag="zp")
                st["exp"] = exp_b
                st["zp"] = zparts
                for q in range(NQ):
                    t0 = q * 512
                    ths = []
                    for ac in range(NA):
                        ps = psA.tile([128, 512], FP32, tag="main")
                        for ec in range(NE):
                            nc.tensor.matmul(
                                ps[:],
                                lhsT=wenc_sb[ec][:, ac * 128 : (ac + 1) * 128],
                                rhs=xqs[q][:, :, ec, :],
                                start=(ec == 0),
                                stop=False,
                            )
                        nc.tensor.matmul(
                            ps[:],
                            lhsT=mext[:, ac * 128 : (ac + 1) * 128],
                            rhs=xs_p[:, q, :, :],
                            start=False,
                            stop=True,
                        )
                        th = thp.tile([128, 512], BF16, tag="tanh")
                        nc.scalar.activation(th[:], ps[:], AF.Tanh)
                        ths.append(th)
                    en_ps = psC.tile([1, 512], FP32, tag="small")
                    for ac in range(NA):
                        nc.tensor.matmul(
                            en_ps[:],
                            lhsT=vdT[ac][:, 0:1],
                            rhs=ths[ac][:],
                            start=(ac == 0),
                            stop=(ac == NA - 1),
                        )
                    if q < 3:
                        nc.scalar.activation(
                            exp_b[0:1, t0 : t0 + 512],
                            en_ps[:],
                            AF.Exp,
                            accum_out=zparts[0:1, q : q + 1],
                        )
                    else:
                        env = en_ps[0:1, :].rearrange("o (j f) -> o j f", j=4)
                        nc.scalar.activation(
                            expv[:, 3, :, 0:116],
                            env[:, :, 0:116],
                            AF.Exp,
                            accum_out=zparts[0:1, q : q + 1],
                        )

            def emit_softmax(b, st):
                exp_b, zparts = st["exp"], st["zp"]
                zsum = rows.tile([1, 1], FP32, tag="zs")
                nc.vector.tensor_reduce(zsum[:], zparts[:], AX.X, ALU.add)
                recip = rows.tile([1, 1], FP32, tag="rc")
                nc.vector.reciprocal(recip[:], zsum[:])
                expv = st["expv"]
                for q in range(NQ):
                    t0 = q * 512
                    if q < 3:
                        sl = exp_b[0:1, t0 : t0 + 512]
                    else:
                        sl = expv[:, 3, :, 0:116]
                    nc.scalar.activation(sl, sl, AF.Copy, scale=recip[:])
                # un-permute to t order (+ cast to f32) and write out
                attn32 = rows.tile([1, TP], FP32, tag="attn32", bufs=2)
                perm_in = exp_b[0:1, :].rearrange("o (q j f) -> o q f j", q=4, j=4)
                nc.vector.tensor_copy(
                    attn32[:].rearrange("o (q f j) -> o q f j", q=4, j=4), perm_in
                )
                nc.scalar.dma_start(out_attn[b : b + 1, :], attn32[0:1, 0:T])

            def emit_context(b, st):
                xqs, exp_b = st["xqs"], st["exp"]
                ctx_acc = ctxp.tile([128, 16], FP32, tag="ctxacc")
                for q in range(NQ):
                    t0 = q * 512
                    bc_ps = psB.tile([128, 512], FP32, tag="bcast")
                    nc.tensor.matmul(
                        bc_ps[:],
                        lhsT=ones_bf[:],
                        rhs=exp_b[0:1, t0 : t0 + 512],
                        start=True,
                        stop=True,
                    )
                    for ec in range(NE):
                        nc.vector.scalar_tensor_tensor(
                            out=junk[:],
                            in0=xqs[q][:, :, ec, :],
                            scalar=1.0,
                            in1=bc_ps[:],
                            op0=ALU.mult,
                            op1=ALU.mult,
                            accum_out=ctx_acc[:, ec * 4 + q : ec * 4 + q + 1],
                        )
                nc.vector.tensor_reduce(
                    ctx_all[:, b * 4 : (b + 1) * 4],
                    ctx_acc[:].rearrange("p (a g) -> p a g", g=4),
                    AX.X,
                    ALU.add,
                )

            # pipeline: prefetch loads one batch ahead; softmax+context of
            # batch b run right after main(b) so PE's bcast matmuls are not
            # queued behind main(b+1), and DVE context work overlaps it.
            states = {0: emit_enc_loads(0)}
            emit_xpad_fill()
            emit_xs_loads(0, states[0])
            emit_w_loads()
            emit_qm(0, states[0])
            states[1] = emit_enc_loads(1)
            emit_xs_loads(1, states[1])
            emit_qm(1, states[1])
            for b in range(BL):
                if b + 2 < BL:
                    states[b + 2] = emit_enc_loads(b + 2)
                    emit_xs_loads(b + 2, states[b + 2])
                    emit_qm(b + 2, states[b + 2])
                emit_main(b, states[b])
                emit_softmax(b, states[b])
                emit_context(b, states[b])

            # context out: transpose [128 e, 16 (b,ec)] -> [16, 128]
            ctxT = wts.tile([32, 128], FP32, tag="ctxT")
            for bi in range(4):
                nc.vector.transpose(
                    ctxT[0:32, 32 * bi : 32 * (bi + 1)],
                    ctx_all[32 * bi : 32 * (bi + 1), 0:32],
                )
            nc.scalar.dma_start(out_ctx[:, :], ctxT[0 : BL * NE, :])

    nc.compile()
    return nc


_NC = None


def _get_nc():
    global _NC
    if _NC is None:
        _NC = build_nc()
    return _NC


def kernel(**inputs):
    nc = _get_nc()
    full = {k: np.ascontiguousarray(np.asarray(v, np.float32)) for k, v in inputs.items()}
    in_maps = []
    for i in range(NCORES):
        sl = slice(i * BL, (i + 1) * BL)
        m = {
            "encoder_states": full["encoder_states"][sl],
            "decoder_outputs": full["decoder_outputs"][sl],
            "attention_weights_step": full["attention_weights_step"][sl],
        }
        for k in ("W_enc", "b_enc", "W_dec", "b_dec", "conv_w", "conv_b", "W_fil", "b_fil", "v_a"):
            m[k] = full[k]
        in_maps.append(m)
    res = run_bass_kernel_spmd(nc, in_maps, core_ids=list(range(NCORES)))
    attn = np.concatenate([r["out_attn"] for r in res.results], axis=0)
    ctx = np.concatenate(
        [r["out_ctx"].reshape(BL, E)[:, None, :] for r in res.results], axis=0
    )
    return (ctx.astype(np.float32), attn.astype(np.float32))


# revision 25
# speedup vs baseline: 1.7148x; 1.5678x over previous
"""Trainium2 Bass kernel for hybrid location-sensitive attention.

Problem: nn_AttentionMechanism_54752243089428
  keys  = enc @ W_enc + b_enc                       [B,T,A]
  query = dec @ W_dec + b_dec                       [B,1,A]
  conv  = Conv1d(attn_prev, conv_w) + conv_b        [B,10,T]
  cfeat = conv^T @ W_fil + b_fil                    [B,T,A]
  energy= tanh(keys+query+cfeat) @ v_a (+b_va)      [B,T]
  attn  = softmax(energy, axis=T)
  ctx   = sum_t attn * enc                          [B,1,E]
  returns (ctx, attn)

Strategy (8 NeuronCores, data-parallel over batch, 4 batches/core):
 - All big compute in bf16 (rel-err gate is 2e-2; bf16 keeps ~0.5%).
 - Layout: [A/E on partitions, T on free].  enc is cast f32->bf16 during
   DMA (SWDGE) into [t,e] staging tiles, then one xbar DMA-transpose per
   512-t quarter produces xt tiles laid out [p, j, ec, f] (e=128*ec+p,
   t=512*q+128*j+f).  All transposes issue from the Sync engine only
   (concurrent xbar use from two HWDGE engines races on xbar state).
 - Conv is folded into the keys matmul: conv_feat^T = M^T xs where
   M[k,a] = sum_c conv_w[c,k] W_fil[c,a] (precomputed on PE) and
   xs[k,t] = xpad[t+k] is a shifted-window matrix built by one strided
   DMA from a padded copy of attn_prev in scratch DRAM.  Row 0 is ones,
   paired with a per-batch row qb = dec@W_dec + (all biases), so one
   PSUM accumulation group produces keys+query+conv_feat+biases.
 - tanh on ScalarE (PSUM->SBUF), energy = v^T tanh via M=1 matmuls,
   softmax without max-subtraction (|energy| <= sum|v| ~ 11, exp safe),
   exp+sum fused via activation accum_out.  t is processed in four
   512 chunks; t in [2000, 2048) is zero-padded and masked out of exp.
 - context via scalar_tensor_tensor (fused multiply + free-dim reduce)
   on VectorE against a PE-broadcast normalized-attn row.  b_va shifts
   all energies equally -> cancels in softmax -> ignored.
"""

import os
import sys

sys.path.insert(0, "/opt/trn_rl_repo")

KVAR = os.environ.get("KVAR", "full")

import numpy as np

import concourse.bass as bass
import concourse.mybir as mybir
import concourse.tile as tile
from concourse import bacc
from concourse.bass_utils import run_bass_kernel_spmd

FP32 = mybir.dt.float32
BF16 = mybir.dt.bfloat16
ALU = mybir.AluOpType
AF = mybir.ActivationFunctionType
AX = mybir.AxisListType

B, T, E, A = 32, 2000, 512, 512
OUT_CH, TAPS, PADK = 10, 101, 50
NCORES = 8
BL = B // NCORES  # 4 batches per core
KX = TAPS + 1  # ones row + taps
TP = 2048  # padded T (t in [2000,2048) zeroed/masked)
XPW = 2176  # padded attn_prev row width (>= TP + PADK + TAPS)
NE = E // 128  # 4 e-chunks
NA = A // 128  # 4 a-chunks
NQ = 4  # t quarters of 512
TVAL = [512, 512, 512, 464]  # valid t per quarter


def build_nc():
    nc = bacc.Bacc("TRN2", target_bir_lowering=False)

    enc = nc.declare_dram_parameter("encoder_states", [BL, T, E], FP32, isOutput=False)
    dec = nc.declare_dram_parameter("decoder_outputs", [BL, 1, E], FP32, isOutput=False)
    awt = nc.declare_dram_parameter("attention_weights_step", [BL, T], FP32, isOutput=False)
    w_enc = nc.declare_dram_parameter("W_enc", [E, A], FP32, isOutput=False)
    b_enc = nc.declare_dram_parameter("b_enc", [A], FP32, isOutput=False)
    w_dec = nc.declare_dram_parameter("W_dec", [E, A], FP32, isOutput=False)
    b_dec = nc.declare_dram_parameter("b_dec", [A], FP32, isOutput=False)
    conv_w = nc.declare_dram_parameter("conv_w", [OUT_CH, 1, TAPS], FP32, isOutput=False)
    conv_b = nc.declare_dram_parameter("conv_b", [OUT_CH], FP32, isOutput=False)
    w_fil = nc.declare_dram_parameter("W_fil", [OUT_CH, A], FP32, isOutput=False)
    b_fil = nc.declare_dram_parameter("b_fil", [A], FP32, isOutput=False)
    v_a = nc.declare_dram_parameter("v_a", [A], FP32, isOutput=False)

    out_attn = nc.declare_dram_parameter("out_attn", [BL, T], FP32, isOutput=True)
    out_ctx = nc.declare_dram_parameter("out_ctx", [BL * NE, 128], FP32, isOutput=True)

    with tile.TileContext(nc) as tc:
        with (
            tc.tile_pool(name="wts", bufs=1) as wts,
            tc.tile_pool(name="xt", bufs=16) as xtp,
            tc.tile_pool(name="stg", bufs=8) as stgp,
            tc.tile_pool(name="wf", bufs=2) as wfp,
            tc.tile_pool(name="xs", bufs=4) as xsp,
            tc.tile_pool(name="mext", bufs=4) as mxp,
            tc.tile_pool(name="tanh", bufs=8) as thp,
            tc.tile_pool(name="rows", bufs=3) as rows,
            tc.tile_pool(name="ctx", bufs=2) as ctxp,
            tc.tile_pool(name="psA", bufs=3, space="PSUM") as psA,
            tc.tile_pool(name="psB", bufs=2, space="PSUM") as psB,
            tc.tile_pool(name="psC", bufs=1, space="PSUM") as psC,
            tc.tile_pool(name="psTr", bufs=2, space="PSUM") as psTr,
            tc.tile_pool(name="dram", bufs=1, space="DRAM") as dramp,
        ):
            # ---------------- weights / constants ----------------
            # small loads first so the critical-path chain (vd32 -> vdT ->
            # qm -> mext, M, biases) is not queued behind megabyte loads
            wfil_sb = wts.tile([OUT_CH, A], BF16, tag="wfil")
            nc.gpsimd.dma_start(wfil_sb[:], w_fil[:, :])
            cw_sb = wts.tile([OUT_CH, KX], BF16, tag="cw")
            nc.vector.memset(cw_sb[:], 0.0)
            nc.gpsimd.dma_start(cw_sb[:, 1 : 1 + TAPS], conv_w[:, 0, :])
            cb_sb = wts.tile([OUT_CH, 1], BF16, tag="cb")
            nc.gpsimd.dma_start(cb_sb[:], conv_b.rearrange("(p o) -> p o", o=1))

            ber = wts.tile([1, A], FP32, tag="ber")
            nc.sync.dma_start(ber[:], b_enc[None, :])
            bdr = wts.tile([1, A], FP32, tag="bdr")
            nc.sync.dma_start(bdr[:], b_dec[None, :])
            bfr = wts.tile([1, A], FP32, tag="bfr")
            nc.sync.dma_start(bfr[:], b_fil[None, :])

            # v_a and decoder outputs, transposed to column layout via DVE
            vd32 = wts.tile([32, E], BF16, tag="vd32")
            nc.vector.memset(vd32[:], 0.0)
            nc.gpsimd.dma_start(vd32[0:1, :], v_a[None, :])
            nc.gpsimd.dma_start(vd32[8 : 8 + BL, :], dec[:, 0, :])

            wenc_sb = []
            wdec_sb = []

            def emit_w_loads():
                for ec in range(NE):
                    wfd = wfp.tile([128, A], FP32, tag="wf32", name=f"wdf{ec}")
                    nc.sync.dma_start(wfd[:], w_dec[ec * 128 : (ec + 1) * 128, :])
                    t_wd = wts.tile([128, A], BF16, tag=f"wdec{ec}", name=f"wd{ec}")
                    nc.scalar.copy(t_wd[:], wfd[:])
                    wdec_sb.append(t_wd)
                    wfe = wfp.tile([128, A], FP32, tag="wf32", name=f"wef{ec}")
                    nc.sync.dma_start(wfe[:], w_enc[ec * 128 : (ec + 1) * 128, :])
                    t_we = wts.tile([128, A], BF16, tag=f"wenc{ec}", name=f"we{ec}")
                    nc.scalar.copy(t_we[:], wfe[:])
                    wenc_sb.append(t_we)

            vdT = []
            for ec in range(NE):
                t_vdT = wts.tile([128, 32], BF16, tag=f"vdT{ec}")
                for bi in range(4):
                    nc.vector.transpose(
                        t_vdT[32 * bi : 32 * (bi + 1), :],
                        vd32[:, ec * 128 + 32 * bi : ec * 128 + 32 * (bi + 1)],
                    )
                vdT.append(t_vdT)

            ones_bf = wts.tile([1, 128], BF16, tag="ones")
            nc.vector.memset(ones_bf[:], 1.0)
            from concourse.masks import make_identity
            ident = wts.tile([128, 128], BF16, tag="ident")
            make_identity(nc, ident[:])

            # M = conv_w^T @ W_fil (row 0 zero); cbW = conv_b @ W_fil
            m_ps = psA.tile([KX, A], FP32, tag="main")
            nc.tensor.matmul(m_ps[:], lhsT=cw_sb[:], rhs=wfil_sb[:], start=True, stop=True)
            m_base = wts.tile([KX, A], BF16, tag="mbase")
            nc.scalar.copy(m_base[:], m_ps[:])
            cbw_ps = psC.tile([1, A], FP32, tag="small")
            nc.tensor.matmul(cbw_ps[:], lhsT=cb_sb[:], rhs=wfil_sb[:], start=True, stop=True)
            bconst = wts.tile([1, A], FP32, tag="bconst")
            nc.vector.tensor_add(bconst[:], ber[:], bdr[:])
            nc.vector.tensor_add(bconst[:], bconst[:], bfr[:])
            nc.vector.tensor_add(bconst[:], bconst[:], cbw_ps[:])

            # ---------------- padded attn_prev in scratch DRAM ----------------
            xpad = dramp.tile([BL, XPW], BF16)
            zedge = wts.tile([BL, 128], BF16, tag="zedge")
            nc.vector.memset(zedge[:], 0.0)
            nc.sync.dma_start(xpad[:, 0:PADK], zedge[:, 0:PADK])
            nc.sync.dma_start(xpad[:, PADK + T : XPW], zedge[:, 0 : XPW - PADK - T])

            def emit_xpad_fill():
                nc.gpsimd.dma_start(xpad[:, PADK : PADK + T], awt[:, :])

            junk = wts.tile([128, 512], BF16, tag="junk")
            ctx_all = wts.tile([128, 32], FP32, tag="ctxall")
            nc.vector.memset(ctx_all[:], 0.0)

            # ---------------- per-batch stages ----------------
            def emit_enc_loads(b):
                """enc -> bf16 staging (SWDGE cast) -> PE transpose blocks
                (via identity matmul) -> PSUM -> copy to xt tiles.
                xt[p, j, ec, f] = enc[t=512q+128j+f, e=128ec+p]."""
                xqs = []
                stgs = []
                for q in range(NQ):
                    stg = stgp.tile([128, 4, 512], BF16, tag="stg", name=f"stg{b}_{q}")
                    if q < 3:
                        nc.gpsimd.dma_start(
                            stg[:],
                            enc[b, q * 512 : (q + 1) * 512, :].rearrange(
                                "(j p) e -> p j e", p=128
                            ),
                        )
                    else:
                        nc.vector.memset(stg[64:128, 3, :], 0.0)
                        nc.gpsimd.dma_start(
                            stg[:, 0:3, :],
                            enc[b, 1536:1920, :].rearrange("(j p) e -> p j e", p=128),
                        )
                        nc.gpsimd.dma_start(stg[0:80, 3, :], enc[b, 1920:2000, :])
                    stgs.append(stg)
                for q in range(NQ):
                    stg = stgs[q]
                    xq = xtp.tile([128, 4, NE, 128], BF16, tag="xt", name=f"xt_b{b}q{q}")
                    for h in range(2):
                        ptr = psTr.tile([128, 1024], BF16, tag="tr", name=f"tr{b}_{q}_{h}")
                        for jj in range(2):
                            j = 2 * h + jj
                            for ec in range(NE):
                                nc.tensor.transpose(
                                    ptr[:, (jj * 4 + ec) * 128 : (jj * 4 + ec + 1) * 128],
                                    stg[:, j, ec * 128 : (ec + 1) * 128],
                                    ident[:],
                                )
                        dst = xq[:, 2 * h : 2 * h + 2, :, :].rearrange(
                            "p j c f -> p (j c f)"
                        )
                        if h == 0:
                            nc.scalar.copy(dst, ptr[:])
                        else:
                            nc.vector.tensor_copy(dst, ptr[:])
                    xqs.append(xq)
                return {"xqs": xqs}

            def emit_xs_loads(b, st):
                """shifted-window matrix via one strided DMA from xpad."""
                xs = xsp.tile([KX, TP], BF16, tag="xs", name=f"xs{b}", bufs=4)
                base = xpad[:, :]
                win = bass.AP(base.tensor, base.offset + b * XPW, [[1, TAPS], [1, TP]])
                nc.sync.dma_start(xs[1 : 1 + TAPS, :], win)
                nc.vector.memset(xs[0:1, :], 1.0)
                st["xs"] = xs

            def emit_qm(b, st):
                qm_ps = psC.tile([1, A], FP32, tag="small", name=f"qm{b}")
                for ec in range(NE):
                    nc.tensor.matmul(
                        qm_ps[:],
                        lhsT=vdT[ec][:, 8 + b : 9 + b],
                        rhs=wdec_sb[ec][:],
                        start=(ec == 0),
                        stop=(ec == NE - 1),
                    )
                mext = mxp.tile([KX, A], BF16, tag="mext", name=f"mext{b}")
                nc.vector.tensor_copy(mext[:], m_base[:])
                nc.vector.scalar_tensor_tensor(
                    out=mext[0:1, :],
                    in0=qm_ps[:],
                    scalar=1.0,
                    in1=bconst[:],
                    op0=ALU.mult,
                    op1=ALU.add,
                )
                st["mext"] = mext

            def emit_main(b, st):
                """keys+conv+bias matmuls, tanh, energy, exp(+partial sums)."""
                xqs, xs, mext = st["xqs"], st["xs"], st["mext"]
                exp_b = rows.tile([1, TP], BF16, tag="exp", bufs=2)
                nc.vector.memset(exp_b[0:1, T:TP], 0.0)
                zparts = rows.tile([1, 4], FP32, tag="zp")
                st["exp"] = exp_b
                st["zp"] = zparts
                for q in range(NQ):
                    t0 = q * 512
                    ths = []
                    for ac in range(NA):
                        ps = psA.tile([128, 512], FP32, tag="main")
                        for ec in range(NE):
                            nc.tensor.matmul(
                                ps[:],
                                lhsT=wenc_sb[ec][:, ac * 128 : (ac + 1) * 128],
                                rhs=xqs[q][:, :, ec, :],
                                start=(ec == 0),
                                stop=False,
                            )
                        nc.tensor.matmul(
                            ps[:],
                            lhsT=mext[:, ac * 128 : (ac + 1) * 128],
                            rhs=xs[:, t0 : t0 + 512],
                            start=False,
                            stop=True,
                        )
                        th = thp.tile([128, 512], BF16, tag="tanh")
                        nc.scalar.activation(th[:], ps[:], AF.Tanh)
                        ths.append(th)
                    en_ps = psC.tile([1, 512], FP32, tag="small")
                    for ac in range(NA):
                        nc.tensor.matmul(
                            en_ps[:],
                            lhsT=vdT[ac][:, 0:1],
                            rhs=ths[ac][:],
                            start=(ac == 0),
                            stop=(ac == NA - 1),
                        )
                    tv = TVAL[q]
                    nc.scalar.activation(
                        exp_b[0:1, t0 : t0 + tv],
                        en_ps[0:1, 0:tv],
                        AF.Exp,
                        accum_out=zparts[0:1, q : q + 1],
                    )

            def emit_softmax(b, st):
                exp_b, zparts = st["exp"], st["zp"]
                zsum = rows.tile([1, 1], FP32, tag="zs")
                nc.vector.tensor_reduce(zsum[:], zparts[:], AX.X, ALU.add)
                recip = rows.tile([1, 1], FP32, tag="rc")
                nc.vector.reciprocal(recip[:], zsum[:])
                for q in range(NQ):
                    t0 = q * 512
                    sl = exp_b[0:1, t0 : t0 + TVAL[q]]
                    nc.scalar.activation(sl, sl, AF.Copy, scale=recip[:])
                attn32 = rows.tile([1, T], FP32, tag="attn32", bufs=2)
                nc.vector.tensor_copy(attn32[:], exp_b[0:1, 0:T])
                nc.sync.dma_start(out_attn[b : b + 1, :], attn32[:])

            def emit_context(b, st):
                xqs, exp_b = st["xqs"], st["exp"]
                ctx_acc = ctxp.tile([128, 16], FP32, tag="ctxacc")
                for q in range(NQ):
                    t0 = q * 512
                    bc_ps = psB.tile([128, 512], FP32, tag="bcast")
                    nc.tensor.matmul(
                        bc_ps[:],
                        lhsT=ones_bf[:],
                        rhs=exp_b[0:1, t0 : t0 + 512],
                        start=True,
                        stop=True,
                    )
                    for ec in range(NE):
                        nc.vector.scalar_tensor_tensor(
                            out=junk[:],
                            in0=xqs[q][:, :, ec, :],
                            scalar=1.0,
                            in1=bc_ps[:],
                            op0=ALU.mult,
                            op1=ALU.mult,
                            accum_out=ctx_acc[:, ec * 4 + q : ec * 4 + q + 1],
                        )
                nc.vector.tensor_reduce(
                    ctx_all[:, b * 4 : (b + 1) * 4],
                    ctx_acc[:].rearrange("p (a g) -> p a g", g=4),
                    AX.X,
                    ALU.add,
                )

            # pipeline: prefetch loads one batch ahead; softmax+context of
            # batch b run right after main(b) so PE's bcast matmuls are not
            # queued behind main(b+1), and DVE context work overlaps it.
            states = {0: emit_enc_loads(0)}
            emit_xpad_fill()
            emit_xs_loads(0, states[0])
            emit_w_loads()
            emit_qm(0, states[0])
            states[1] = emit_enc_loads(1)
            emit_xs_loads(1, states[1])
            emit_qm(1, states[1])
            for b in range(BL):
                if b + 2 < BL:
                    states[b + 2] = emit_enc_loads(b + 2)
                    emit_xs_loads(b + 2, states[b + 2])
                    emit_qm(b + 2, states[b + 2])
                emit_main(b, states[b])
                emit_softmax(b, states[b])
                emit_context(b, states[b])

            # context out: transpose [128 e, 16 (b,ec)] -> [16, 128]
            ctxT = wts.tile([32, 128], FP32, tag="ctxT")
            for bi in range(4):
                nc.vector.transpose(
                    ctxT[0:32, 32 * bi : 32 * (bi + 1)],
                    ctx_all[32 * bi : 32 * (bi + 1), 0:32],
                )
            nc.sync.dma_start(out_ctx[:, :], ctxT[0 : BL * NE, :])

    nc.compile()
    return nc


_NC = None


def _get_nc():
    global _NC
    if _NC is None:
        _NC = build_nc()
    return _NC


def kernel(**inputs):
    nc = _get_nc()
    full = {k: np.ascontiguousarray(np.asarray(v, np.float32)) for k, v in inputs.items()}
    in_maps = []
    for i in range(NCORES):
        sl = slice(i * BL, (i + 1) * BL)
        m = {
            "encoder_states": full["encoder_states"][sl],
            "decoder_outputs": full["decoder_outputs"][sl],
            "attention_weights_step": full["attention_weights_step"][sl],
        }
        for k in ("W_enc", "b_enc", "W_dec", "b_dec", "conv_w", "conv_b", "W_fil", "b_fil", "v_a"):
            m[k] = full[k]
        in_maps.append(m)
    res = run_bass_kernel_spmd(nc, in_maps, core_ids=list(range(NCORES)))
    attn = np.concatenate([r["out_attn"] for r in res.results], axis=0)
    ctx = np.concatenate(
        [r["out_ctx"].reshape(BL, E)[:, None, :] for r in res.results], axis=0
    )
    return (ctx.astype(np.float32), attn.astype(np.float32))
